# revision 2
# baseline (speedup 1.0000x reference)
"""AggregationLoss Trainium2 kernel.

Algorithm (per image, per core; 2 images per core, 8 cores):
  pass 1: segment sums of (pred x4, kernels_mask) over kernel_labels via
          bf16 one-hot (tensor_scalar is_equal) + per-column PE matmul
          accumulating [5, 38] in PSUM.
  table:  G[s,c] = sums[s,c] / (card[s]+1), G[:,0] = 0, broadcast to a
          [128, 38] per-16-partition-group component table.
  pass 2: per-pixel gather of G[label] via gpsimd indirect_copy (component
          per partition trick), DMA rearrange to planar, then
          diff = pred*rmask - G[l], norm^2, d = relu(sqrt - 0.5),
          D = ln(d^2 + 1), reduce.
Output per core: [1, 2] f32 = [sum_D_partial, max_label_of_last_image].
Host combines: total = sum(partials) / nk.
"""

import sys
import functools
from contextlib import ExitStack

import numpy as np

for _p in ("/opt/trn_rl_repo",):
    if _p not in sys.path:
        sys.path.insert(0, _p)

# ---- problem constants (hardcoded per contract) ----
B, C, H, W = 16, 4, 736, 736
HW = H * W            # 541696
P = 128
NCORES = 8
IPC = B // NCORES     # images per core = 2
T_RAW = HW // P       # 4232
T_FULL = 4480         # padded free size (pad pixels: label 0, data 0 -> D = 0)
CH_FULL = 448         # chunk size (10 chunks per image), multiple of 16
S = 38                # one-hot / table width (labels 0..36, col 37 zero)
SIGMA = 0.5


def build_nc(T, CH, ipc, dve_onehot_lo=0):
    """Build the per-core Bass program.

    T: free-dim size per partition per image (pixels = P*T per image)
    CH: chunk size (T % CH == 0)
    dve_onehot_lo: segments [0, dve_onehot_lo) are built on ACT (2-op
        abs/relu trick), rest on DVE.  0 = all on DVE.
    """
    import concourse.bass as bass
    import concourse.bacc as bacc
    import concourse.mybir as mybir
    import concourse.tile as tile

    fp32 = mybir.dt.float32
    bf16 = mybir.dt.bfloat16
    i32 = mybir.dt.int32
    u16 = mybir.dt.uint16
    AF = mybir.ActivationFunctionType
    ALU = mybir.AluOpType

    NCH = T // CH
    assert T % CH == 0
    CHP = CH + (CH % 2)   # padded (even) chunk width for one-hot 4x mode

    nc = bacc.Bacc("TRN2", target_bir_lowering=False, debug=False)

    pred_d = nc.dram_tensor("pred", [ipc * C, HW_of(T)], fp32, kind="ExternalInput")
    rmask_d = nc.dram_tensor("rmask", [ipc, HW_of(T)], fp32, kind="ExternalInput")
    km_d = nc.dram_tensor("km", [ipc, HW_of(T)], fp32, kind="ExternalInput")
    lab_d = nc.dram_tensor("labels", [ipc, HW_of(T)], i32, kind="ExternalInput")
    lsh_d = nc.dram_tensor("labsh", [ipc, HW_of(T)], u16, kind="ExternalInput")
    out_d = nc.dram_tensor("out", [1, 2], fp32, kind="ExternalOutput")

    pred_r = pred_d.ap().rearrange("a (p t) -> a p t", p=P)
    pred4_r = pred_d.ap().rearrange("(i c) (p t) -> i p c t", c=C, p=P)
    rmask_r = rmask_d.ap().rearrange("a (p t) -> a p t", p=P)
    km_r = km_d.ap().rearrange("a (p t) -> a p t", p=P)
    lab_r = lab_d.ap().rearrange("a (p t) -> a p t", p=P)
    lsh_r = lsh_d.ap().rearrange("a (p t) -> a p t", p=P)

    with tile.TileContext(nc) as tc, ExitStack() as ctx:
        resid = ctx.enter_context(tc.tile_pool(name="resid", bufs=1))
        io = ctx.enter_context(tc.tile_pool(name="io", bufs=2))
        oh = ctx.enter_context(tc.tile_pool(name="oh", bufs=2))
        d5p = ctx.enter_context(tc.tile_pool(name="d5p", bufs=2))
        gp = ctx.enter_context(tc.tile_pool(name="gp", bufs=2))
        ps = ctx.enter_context(tc.tile_pool(name="ps", bufs=2, space="PSUM"))
        pst = ctx.enter_context(tc.tile_pool(name="pst", bufs=1, space="PSUM"))
        sm = ctx.enter_context(tc.tile_pool(name="sm", bufs=2))

        # ---- residents ----
        labsh = [resid.tile([P, T], u16, name=f"labsh{i}", tag=f"labsh{i}") for i in range(ipc)]
        tab = [resid.tile([P, S], bf16, name=f"tab{i}", tag=f"tab{i}") for i in range(ipc)]
        rd = resid.tile([P, T], bf16, tag="rd")          # relu(norm - sigma), per image
        acc = resid.tile([P, NCH * ipc], fp32, tag="acc")
        ones = resid.tile([P, 1], fp32, tag="ones")
        nc.gpsimd.memset(ones[:], 1.0)
        bneg = resid.tile([P, 1], fp32, tag="bneg")
        nc.gpsimd.memset(bneg[:], -SIGMA)

        psums = []
        # =========== PASS 1 (both images) ===========
        for img in range(ipc):
            psum = ps.tile([5, S], fp32, tag="sums")
            psums.append(psum)
            for k in range(NCH):
                sl = slice(k * CH, (k + 1) * CH)
                lab32 = io.tile([P, CH], i32, tag="lab32")
                nc.sync.dma_start(lab32[:], lab_r[img, :, sl])
                labbf = io.tile([P, CHP], bf16, tag="labbf")
                if CHP != CH:
                    nc.vector.memset(labbf[:, CH:CHP], 0.0)
                nc.vector.tensor_copy(labbf[:, :CH], lab32[:])
                nc.sync.dma_start(labsh[img][:, sl], lsh_r[img, :, sl])

                data5 = d5p.tile([P, 5, CH], bf16, tag="data5")
                p4 = io.tile([P, C, CH], fp32, tag="p4")
                nc.sync.dma_start(p4[:], pred4_r[img, :, :, sl])
                nc.scalar.copy(data5[:, 0:C, :], p4[:])
                t32 = io.tile([P, CH], fp32, tag="p132")
                nc.sync.dma_start(t32[:], km_r[img, :, sl])
                nc.vector.tensor_copy(data5[:, 4, :], t32[:])

                # one-hot [P, S, CHP]
                O = oh.tile([P, S, CHP], bf16, tag="onehot")
                for s in range(S):
                    if s < dve_onehot_lo:
                        # ACT: relu(1 - |lab - s|)
                        tmp = io.tile([P, CHP], bf16, tag="ohtmp")
                        nc.scalar.activation(tmp[:], labbf[:], AF.Abs, bias=float(-s))
                        nc.scalar.activation(O[:, s, :], tmp[:], AF.Relu,
                                             bias=1.0, scale=-1.0)
                    else:
                        nc.vector.tensor_scalar(O[:, s, :], labbf[:], float(s), None,
                                                op0=ALU.is_equal)

                for t in range(CH):
                    nc.tensor.matmul(
                        psum[:], data5[:, :, t], O[:, :, t],
                        start=(k == 0 and t == 0),
                        stop=(k == NCH - 1 and t == CH - 1),
                    )

            # ---- table build (all ops at partition base 0) ----
            sums_sb = resid.tile([5, S], fp32, name=f"sums_sb{img}", tag=f"sums_sb{img}")
            nc.vector.tensor_copy(sums_sb[:], psum[:])
            card0 = resid.tile([1, S], fp32, name=f"card0{img}", tag=f"card0{img}")
            nc.sync.dma_start(card0[:], sums_sb[4:5, :])
            denom = resid.tile([1, S], fp32, name=f"denom{img}", tag=f"denom{img}")
            nc.vector.tensor_scalar_add(denom[:], card0[:], 1.0)
            recip = resid.tile([1, S], fp32, name=f"recip{img}", tag=f"recip{img}")
            nc.vector.reciprocal(recip[:], denom[:])
            rec4 = resid.tile([4, S], fp32, name=f"rec4{img}", tag=f"rec4{img}")
            for c in range(C):
                nc.sync.dma_start(rec4[c:c + 1, :], recip[:])
            Gf = resid.tile([4, S], fp32, name=f"Gf{img}", tag=f"Gf{img}")
            nc.vector.tensor_mul(Gf[:], sums_sb[0:4, :], rec4[:])
            nc.gpsimd.memset(Gf[:, 0:1], 0.0)   # label 0 -> G = 0
            Gbf = resid.tile([4, S], bf16, name=f"Gbf{img}", tag=f"Gbf{img}")
            nc.vector.tensor_copy(Gbf[:], Gf[:])
            nc.gpsimd.memset(tab[img][:], 0.0)
            for g in range(P // 16):
                nc.sync.dma_start(tab[img][16 * g:16 * g + 4, :], Gbf[:])

        # =========== PASS 2 ===========
        from concourse.tile_rust import add_dep_helper
        prev_redma = {0: [], 1: []}
        prev_cons = {0: [], 1: []}
        kk = 0
        for img in range(ipc):
            # ---- phase A: gather + norm + relu(sqrt - sigma) ----
            for k in range(NCH):
                sl = slice(k * CH, (k + 1) * CH)
                outsp = gp.tile([P, 16 * CH], bf16, tag="outsp")
                # ISA limits the per-instruction dst elem count: split into 4
                # m'-block-aligned sub-gathers (SPW multiple of CH//16).
                NSP = 8
                SPW = CH // NSP
                assert SPW * 16 <= 1024
                gths = []
                for j in range(NSP):
                    gth = nc.gpsimd.indirect_copy(
                        outsp[:, j * 16 * SPW:(j + 1) * 16 * SPW], tab[img][:],
                        labsh[img][:, k * CH + j * SPW:k * CH + (j + 1) * SPW],
                        True)
                    gths.append(gth)
                    # WAR/WAW vs under-tracked rearrange DMAs two chunks back
                    for d in prev_redma[kk % 2]:
                        add_dep_helper(gth.ins, d.ins, reason="outsp slot reuse")
                # out[16g+c, m*CH + t] = G_c(pixel(16g+m, k*CH+t)); rearrange
                # to planar gpl[16g+m, c, t].
                gpl = gp.tile([P, C, CH], bf16, tag="gpl")
                redma = []
                for c in range(C):
                    for g in range(P // 16):
                        sap = outsp[16 * g + c:16 * g + c + 1, :].rearrange(
                            "p (n t) -> p n t", t=CH)
                        d = nc.scalar.dma_start(gpl[16 * g:16 * (g + 1), c, :], sap)
                        for gth in gths:
                            add_dep_helper(d.ins, gth.ins, reason="rearr after gather")
                        for pc in prev_cons[kk % 2]:
                            add_dep_helper(d.ins, pc.ins, reason="gpl slot reuse")
                        redma.append(d)
                prev_redma[kk % 2] = redma

                rm32 = io.tile([P, CH], fp32, tag="rm32")
                nc.sync.dma_start(rm32[:], rmask_r[img, :, sl])
                rmbf = io.tile([P, CH], bf16, tag="rmbf")
                nc.scalar.copy(rmbf[:], rm32[:])

                p4 = io.tile([P, C, CH], fp32, tag="p4")
                nc.sync.dma_start(p4[:], pred4_r[img, :, :, sl])
                pbf4 = io.tile([P, C, CH], bf16, tag="pbf4")
                nc.scalar.copy(pbf4[:], p4[:])

                cons = []
                nsq = io.tile([P, CH], fp32, tag="nsq")
                sq0 = io.tile([P, CH], fp32, tag="sq0")
                sq1 = io.tile([P, CH], fp32, tag="sq1")
                for c in range(C):
                    fp = io.tile([P, CH], bf16, tag="fp")
                    nc.vector.tensor_mul(fp[:], pbf4[:, c, :], rmbf[:])
                    df = io.tile([P, CH], bf16, tag="df")
                    dfi = nc.vector.tensor_sub(df[:], fp[:], gpl[:, c, :])
                    for d in redma:
                        add_dep_helper(dfi.ins, d.ins, reason="df after rearrange")
                    cons.append(dfi)
                    if c == 0:
                        nc.scalar.square(nsq[:], df[:])
                    elif c == 1:
                        nc.scalar.square(sq0[:], df[:])
                    elif c == 2:
                        nc.scalar.square(sq1[:], df[:])
                        nc.vector.tensor_add(nsq[:], nsq[:], sq0[:])
                    else:
                        nc.scalar.square(sq0[:], df[:])
                        nc.vector.tensor_add(sq1[:], sq1[:], sq0[:])
                        nc.vector.tensor_add(nsq[:], nsq[:], sq1[:])
                nc.scalar.sqrt(nsq[:], nsq[:])
                nc.scalar.activation(rd[:, sl], nsq[:], AF.Relu, bias=bneg[:])
                prev_cons[kk % 2] = cons
                kk += 1

            # ---- phase B: D = ln(d^2 + 1), reduce ----
            for k in range(NCH):
                sl = slice(k * CH, (k + 1) * CH)
                d2 = io.tile([P, CH], fp32, tag="d2")
                nc.scalar.square(d2[:], rd[:, sl])
                nc.scalar.activation(d2[:], d2[:], AF.Ln, bias=1.0)
                nc.vector.tensor_reduce(
                    acc[:, img * NCH + k: img * NCH + k + 1], d2[:],
                    axis=mybir.AxisListType.X, op=ALU.add)

        # =========== finale ===========
        accs = sm.tile([P, 1], fp32, tag="accs")
        nc.vector.tensor_reduce(accs[:], acc[:], axis=mybir.AxisListType.X,
                                op=ALU.add)
        tot = pst.tile([1, 1], fp32, tag="tot")
        nc.tensor.matmul(tot[:], ones[:], accs[:], start=True, stop=True)

        # num_kernel = max(labels of last local image)
        nkx = sm.tile([P, 1], u16, tag="nkx")
        nc.vector.tensor_reduce(nkx[:], labsh[ipc - 1][:], axis=mybir.AxisListType.X,
                                op=ALU.max)
        nkf = sm.tile([P, 1], fp32, tag="nkf")
        nc.vector.tensor_copy(nkf[:], nkx[:])
        nk1 = sm.tile([1, 1], fp32, tag="nk1")
        nc.gpsimd.tensor_reduce(nk1[:], nkf[:], axis=mybir.AxisListType.C,
                                op=ALU.max)

        outsb = sm.tile([1, 2], fp32, tag="outsb")
        nc.vector.tensor_copy(outsb[:, 0:1], tot[:])
        nc.vector.tensor_copy(outsb[:, 1:2], nk1[:])
        nc.sync.dma_start(out_d.ap(), outsb[:])

    nc.compile()
    return nc


def HW_of(T):
    return P * T


@functools.lru_cache(maxsize=2)
def _get_full_nc():
    return build_nc(T_FULL, CH_FULL, IPC)


def _pad_T(a, T):
    """[N, HW] -> [N, P, T] zero-padded along the per-partition axis."""
    n = a.shape[0]
    out = np.zeros((n, P, T), dtype=a.dtype)
    out[:, :, :T_RAW] = a.reshape(n, P, T_RAW)
    return out


def _make_labsh(lab_pad, CH):
    """lab_pad [ipc, P, T] int -> shuffled uint16 idx layout for the gather.

    labsh[i, 16g+r, k*CH + m*K16 + w] = lab_pad[i, 16g+m, k*CH + 16w + r]
    """
    ipc, _, T = lab_pad.shape
    K16 = CH // 16
    NCH = T // CH
    A = lab_pad.reshape(ipc, 8, 16, NCH, K16, 16)      # [i, g, m, k, w, r]
    A = A.transpose(0, 1, 5, 3, 2, 4)                  # [i, g, r, k, m, w]
    return np.ascontiguousarray(A.reshape(ipc, P, T).astype(np.uint16))


def kernel(pred_similarities, regions_mask, kernels_mask, kernel_labels):
    from concourse import bass_utils

    pred = np.asarray(pred_similarities, dtype=np.float32).reshape(B * C, HW)
    rmask = np.asarray(regions_mask, dtype=np.float32).reshape(B, HW)
    km = np.asarray(kernels_mask, dtype=np.float32).reshape(B, HW)
    lab = np.asarray(kernel_labels, dtype=np.int32).reshape(B, HW)

    T = T_FULL
    in_maps = []
    for i in range(NCORES):
        s = slice(i * IPC, (i + 1) * IPC)
        sc = slice(i * IPC * C, (i + 1) * IPC * C)
        lab_pad = _pad_T(lab[s], T)
        in_maps.append({
            "pred": _pad_T(pred[sc], T).reshape(IPC * C, P * T),
            "rmask": _pad_T(rmask[s], T).reshape(IPC, P * T),
            "km": _pad_T(km[s], T).reshape(IPC, P * T),
            "labels": lab_pad.reshape(IPC, P * T),
            "labsh": _make_labsh(lab_pad, CH_FULL).reshape(IPC, P * T),
        })

    nc = _get_full_nc()
    res = bass_utils.run_bass_kernel_spmd(nc, in_maps, core_ids=list(range(NCORES)))
    global LAST_RESULT
    LAST_RESULT = res
    outs = [r["out"] for r in res.results]
    total = float(sum(o[0, 0] for o in outs))
    nk = float(outs[NCORES - 1][0, 1])
    return np.array(total / nk, dtype=np.float32)


# ---------------- development helpers ----------------

def _ref_percore(pred, rmask, km, lab, T):
    """Numpy reference for the per-core program: returns [sum_D, nk]."""
    ipc = lab.shape[0]
    tot = 0.0
    for img in range(ipc):
        x = pred[img * C:(img + 1) * C].astype(np.float64)     # [C, HW]
        r = rmask[img].astype(np.float64)
        k_ = km[img].astype(np.float64)
        l_ = lab[img].astype(np.int64)
        sums = np.zeros((S, C))
        card = np.zeros(S)
        np.add.at(card, l_, k_)
        for c in range(C):
            np.add.at(sums[:, c], l_, x[c])
        G = sums / (card[:, None] + 1.0)
        G[0] = 0.0
        g = G[l_]                                             # [HW, C]
        fp = x * r[None, :]
        d2 = ((fp.T - g) ** 2).sum(1)
        d = np.maximum(np.sqrt(d2) - SIGMA, 0.0)
        tot += np.log(d * d + 1.0).sum()
    return np.array([tot, lab[ipc - 1].max()], dtype=np.float64)


def _selftest_sim(T=64, CH=32):
    from concourse.bass_interp import CoreSim
    rng = np.random.default_rng(0)
    hw = P * T
    ipc = IPC
    pred = rng.standard_normal((ipc * C, hw)).astype(np.float32)
    rmask = rng.random((ipc, hw)).astype(np.float32)
    km = rng.random((ipc, hw)).astype(np.float32)
    lab = rng.integers(0, 37, (ipc, hw)).astype(np.int32)
    labsh = _make_labsh(lab.reshape(ipc, P, T), CH).reshape(ipc, hw)

    nc = build_nc(T, CH, ipc)
    sim = CoreSim(nc, trace=False)
    sim.tensor("pred")[:] = pred
    sim.tensor("rmask")[:] = rmask
    sim.tensor("km")[:] = km
    sim.tensor("labels")[:] = lab
    sim.tensor("labsh")[:] = labsh
    sim.simulate(check_with_hw=False)
    got = np.array(sim.tensor("out")).reshape(2)
    want = _ref_percore(pred, rmask, km, lab, T)
    print("got ", got)
    print("want", want)
    rel = abs(got[0] - want[0]) / abs(want[0])
    print("rel err:", rel)
    assert got[1] == want[1], (got[1], want[1])
    assert rel < 2e-2, rel
    print("SELFTEST PASS")


if __name__ == "__main__":
    _selftest_sim()



# revision 12
# speedup vs baseline: 3.8619x; 3.8619x over previous
"""AggregationLoss Trainium2 kernel — quad-stream design.

Host packs each image (per 16-partition group) into a stream of QUADS:
4 same-label pixels per stream position.  Per core: 2 images.

Device per image:
  pass 1 (position-major): one-hot of quad labels [128 pos, 37] ->
     1088 matmuls accumulating [20, 38] = per-(slot,channel) segment sums
     (each MM contracts 128 quads = 512 pixels).
  table: G[c,s] = sums/(card+1), G[:,0]=0 -> gather table tab[128,38]
     (rows 16g+4j+c = G_c), asq[s] = sum_c G_c^2.
  pass 2 (quad-c-row layout): gpsimd indirect_copy gathers tab per quad
     (one index per 4 pixels); n2 = P2 - 2X + A assembled by 3 PSUM-
     accumulated matmuls with constant +-1/-2 weights over c-rows:
       P2 = sum_c Fp^2, X = sum_c Fp*G, A = sum_c G^2  (Fp = pred*rmask)
     then ACT chain: relu-pack -> sqrt -> d = relu(.-sigma), and later
     D = ln(d^2+1) -> reduce.
  dummy slots in partial quads contribute ln(relu(sqrt(asq)-s)^2+1);
  corrected exactly via host-side dummy counts nd[s].
Output per core: [1, 2] = [sum_D, max label of last local image].
"""

import sys
import functools
from contextlib import ExitStack

import numpy as np

for _p in ("/opt/trn_rl_repo",):
    if _p not in sys.path:
        sys.path.insert(0, _p)

# ---- problem constants (hardcoded per contract) ----
B, C, H, W = 16, 4, 736, 736
HW = H * W              # 541696
P = 128
NCORES = 8
IPC = B // NCORES       # images per core = 2
NG = 8                  # partition groups of 16
GPIX = HW // NG         # pixels per group = 67712
NLAB = 37               # labels 0..36
S = 38                  # table width (cols 0..37, col 37 unused)
SIGMA = 0.5

F = 17408               # quad stream length per group (>= 16956 worst case)
CHC = 512               # pass-2 chunk (positions per chunk)
NCH = F // CHC          # 34
FP = (NG * F) // P      # position-major free size = 1088
NPACK = (NCH + 3) // 4  # packed d blocks (9; last half)


def build_nc(F, CHC, ipc):
    import concourse.bass as bass
    import concourse.bacc as bacc
    import concourse.mybir as mybir
    import concourse.tile as tile

    fp32 = mybir.dt.float32
    f16 = mybir.dt.float16
    u16 = mybir.dt.uint16
    AF = mybir.ActivationFunctionType
    ALU = mybir.AluOpType

    NCH = F // CHC
    FP = NG * F // P
    NPACK = (NCH + 3) // 4
    assert F % 16 == 0 and F % CHC == 0 and CHC % 16 == 0
    assert (NG * F) % P == 0

    nc = bacc.Bacc("TRN2", target_bir_lowering=False, debug=False)

    pdpos_d = nc.dram_tensor("pdpos", [ipc, P * 20 * FP], f16, kind="ExternalInput")
    labpos_d = nc.dram_tensor("labpos", [ipc, P * FP], f16, kind="ExternalInput")
    pdata_d = nc.dram_tensor("pdata", [ipc, P * F], f16, kind="ExternalInput")
    rmq_d = nc.dram_tensor("rmq", [ipc, P * F], f16, kind="ExternalInput")
    glab_d = nc.dram_tensor("glab", [ipc, P * (F // 16)], u16, kind="ExternalInput")
    nd_d = nc.dram_tensor("nd", [ipc, S], fp32, kind="ExternalInput")
    wall_d = nc.dram_tensor("wall", [1, P * 96], f16, kind="ExternalInput")
    wall5_d = nc.dram_tensor("wall5", [1, P * 5], fp32, kind="ExternalInput")
    out_d = nc.dram_tensor("out", [1, 2], fp32, kind="ExternalOutput")

    pdpos_r = pdpos_d.ap().rearrange("i (p x) -> i p x", p=P)
    labpos_r = labpos_d.ap().rearrange("i (p x) -> i p x", p=P)
    pdata_r = pdata_d.ap().rearrange("i (p x) -> i p x", p=P)
    rmq_r = rmq_d.ap().rearrange("i (p x) -> i p x", p=P)
    glab_r = glab_d.ap().rearrange("i (p x) -> i p x", p=P)
    wall_r = wall_d.ap().rearrange("i (p x) -> i p x", p=P)

    with tile.TileContext(nc) as tc, ExitStack() as ctx:
        resid = ctx.enter_context(tc.tile_pool(name="resid", bufs=1))
        ohp = ctx.enter_context(tc.tile_pool(name="ohp", bufs=1))
        pdp = ctx.enter_context(tc.tile_pool(name="pdp", bufs=2))
        io = ctx.enter_context(tc.tile_pool(name="io", bufs=3))
        mid = ctx.enter_context(tc.tile_pool(name="mid", bufs=3))
        pk = ctx.enter_context(tc.tile_pool(name="pk", bufs=2))
        ps1 = ctx.enter_context(tc.tile_pool(name="ps1", bufs=2, space="PSUM"))
        ps2 = ctx.enter_context(tc.tile_pool(name="ps2", bufs=2, space="PSUM"))
        pst = ctx.enter_context(tc.tile_pool(name="pst", bufs=1, space="PSUM"))
        sm = ctx.enter_context(tc.tile_pool(name="sm", bufs=4))

        # ---- constants / residents ----
        wall = resid.tile([P, 96], f16, tag="wall")
        nc.sync.dma_start(wall[:], wall_r[0])
        wall5 = resid.tile([P, 5], fp32, tag="wall5")
        nc.sync.dma_start(wall5[:], wall5_d.ap().rearrange("i (p x) -> i p x", p=P)[0])
        ones = resid.tile([P, 1], fp32, tag="ones")
        nc.gpsimd.memset(ones[:], 1.0)
        bneg = resid.tile([P, 1], fp32, tag="bneg")
        nc.gpsimd.memset(bneg[:], -SIGMA)
        acc = resid.tile([P, ipc], fp32, tag="acc")

        tabs, glabs, asqs, dvs, nds, dres, nkmax = [], [], [], [], [], [], []
        for img in range(ipc):
            tabs.append(resid.tile([P, S], f16, name=f"tab{img}", tag=f"tab{img}"))
            glabs.append(resid.tile([P, F // 16], u16, name=f"glab{img}", tag=f"glab{img}"))
            asqs.append(resid.tile([1, S], fp32, name=f"asq{img}", tag=f"asq{img}"))
            dvs.append(resid.tile([1, S], fp32, name=f"dv{img}", tag=f"dv{img}"))
            nds.append(resid.tile([1, S], fp32, name=f"nd{img}", tag=f"nd{img}"))
            dres.append(resid.tile([P, NPACK * CHC], f16, name=f"dres{img}", tag=f"dres{img}"))
            nkmax.append(resid.tile([P, 1], f16, name=f"nk{img}", tag=f"nk{img}"))

        # ================= PASS 1 (both images) =================
        for img in range(ipc):
            lp = ohp.tile([P, FP], f16, tag="labpos")
            nc.sync.dma_start(lp[:], labpos_r[img])
            nc.sync.dma_start(glabs[img][:], glab_r[img])
            nc.sync.dma_start(nds[img][:], nd_d.ap()[img : img + 1, :])

            # one-hot [P, NLAB(+pad to S), FP]
            O = ohp.tile([P, S, FP], f16, tag="onehot")
            for s in range(NLAB):
                nc.vector.tensor_scalar(O[:, s, :], lp[:], float(s), None,
                                        op0=ALU.is_equal)
            nc.vector.memset(O[:, NLAB:S, :], 0.0)

            # max label (for nk) of this image
            nc.vector.tensor_reduce(nkmax[img][:], lp[:],
                                    axis=mybir.AxisListType.X, op=ALU.max)

            psums = ps1.tile([20, S], fp32, tag="ps1")
            NPD = 4
            PDC = FP // NPD
            for cc in range(NPD):
                pdt = pdp.tile([P, 20 * PDC], f16, tag="pdt")
                nc.sync.dma_start(pdt[:], pdpos_r[img, :, 20 * PDC * cc : 20 * PDC * (cc + 1)])
                for ql in range(PDC):
                    q = cc * PDC + ql
                    nc.tensor.matmul(
                        psums[:], pdt[:, 20 * ql : 20 * ql + 20], O[:, :, q],
                        start=(q == 0), stop=(q == FP - 1),
                    )

            # ---- table build ----
            # sums5 [5, S] = sum over j of psums[5j+c]: tiny matmul with
            # wall cols 96:101 (W5[5j+c, c] = 1).
            sums20 = sm.tile([20, S], fp32, tag="sums20")
            nc.vector.tensor_copy(sums20[:], psums[:])
            ps5 = pst.tile([5, S], fp32, tag="ps5")
            nc.tensor.matmul(ps5[:], wall5[0:20, :], sums20[0:20, :],
                             start=True, stop=True)
            sums5 = sm.tile([5, S], fp32, tag="sums5")
            nc.vector.tensor_copy(sums5[:], ps5[:])

            card0 = sm.tile([1, S], fp32, tag="card0")
            nc.scalar.dma_start(card0[:], sums5[4:5, :])
            denom = sm.tile([1, S], fp32, tag="denom")
            nc.vector.tensor_scalar_add(denom[:], card0[:], 1.0)
            recip = sm.tile([1, S], fp32, tag="recip")
            nc.vector.reciprocal(recip[:], denom[:])
            rec4 = sm.tile([4, S], fp32, tag="rec4")
            for c in range(C):
                nc.scalar.dma_start(rec4[c : c + 1, :], recip[:])
            Gf = sm.tile([4, S], fp32, tag="Gf")
            nc.vector.tensor_mul(Gf[:], sums5[0:4, :], rec4[:])
            nc.gpsimd.memset(Gf[:, 0:1], 0.0)
            G16 = sm.tile([4, S], f16, tag="G16")
            nc.vector.tensor_copy(G16[:], Gf[:])

            # asq[s] = sum_c G_c(s)^2 via ones-matmul
            sqG = sm.tile([4, S], fp32, tag="sqG")
            nc.scalar.square(sqG[:], Gf[:])
            psa = pst.tile([1, S], fp32, tag="psa")
            nc.tensor.matmul(psa[:], ones[0:4, :], sqG[:], start=True, stop=True)
            nc.vector.tensor_copy(asqs[img][:], psa[:])

            # tab[16g+4j+c, s] = G_c(s): replicate [4, S] block 32x
            g16 = sm.tile([16, S], f16, tag="g16")
            for r in range(4):
                nc.scalar.dma_start(g16[4 * r : 4 * r + 4, :], G16[:])
            for g in range(NG):
                nc.scalar.dma_start(tabs[img][16 * g : 16 * g + 16, :], g16[:])

        # ================= PASS 2 (both images) =================
        for img in range(ipc):
            packt = None
            for k in range(NCH):
                sl = slice(k * CHC, (k + 1) * CHC)
                gout = io.tile([P, CHC], f16, tag="gout")
                nc.gpsimd.indirect_copy(
                    gout[:], tabs[img][:],
                    glabs[img][:, k * (CHC // 16) : (k + 1) * (CHC // 16)], True)

                pda = io.tile([P, CHC], f16, tag="pda")
                nc.sync.dma_start(pda[:], pdata_r[img, :, sl])
                rmq = io.tile([P, CHC], f16, tag="rmq")
                nc.sync.dma_start(rmq[:], rmq_r[img, :, sl])

                fpq = mid.tile([P, CHC], f16, tag="fpq")
                nc.vector.tensor_mul(fpq[:], pda[:], rmq[:])
                prod = mid.tile([P, CHC], f16, tag="prod")
                nc.vector.tensor_mul(prod[:], fpq[:], gout[:])
                sq = mid.tile([P, CHC], f16, tag="sq")
                nc.scalar.square(sq[:], fpq[:])
                gsq = mid.tile([P, CHC], f16, tag="gsq")
                nc.scalar.square(gsq[:], gout[:])

                n2 = ps2.tile([32, CHC], fp32, tag="n2")
                nc.tensor.matmul(n2[:], wall[:, 0:32], sq[:], start=True, stop=False)
                nc.tensor.matmul(n2[:], wall[:, 32:64], prod[:], start=False, stop=False)
                nc.tensor.matmul(n2[:], wall[:, 64:96], gsq[:], start=False, stop=True)

                # relu-pack into [128, CHC] (4 chunks per pack)
                cc = k % 4
                if cc == 0:
                    packt = pk.tile([P, CHC], f16, tag="packt")
                nc.scalar.activation(packt[32 * cc : 32 * cc + 32, :], n2[:], AF.Relu)
                if cc == 3 or k == NCH - 1:
                    pb = k // 4
                    rows = 32 * (cc + 1)
                    nrm = pk.tile([P, CHC], f16, tag="nrm")
                    nc.scalar.sqrt(nrm[0:rows, :], packt[0:rows, :])
                    nc.scalar.activation(
                        dres[img][0:rows, pb * CHC : (pb + 1) * CHC],
                        nrm[0:rows, :], AF.Relu, bias=bneg[0:rows, :])
                    if rows < P:
                        nc.vector.memset(
                            dres[img][rows:P, pb * CHC : (pb + 1) * CHC], 0.0)

            # dummy-slot correction, sqrt part: dv = relu(sqrt(asq) - sigma)
            sqa = sm.tile([1, S], fp32, tag="sqa")
            nc.scalar.sqrt(sqa[:], asqs[img][:])
            nc.scalar.activation(dvs[img][:], sqa[:], AF.Relu, bias=bneg[0:1, :])

        # ================= PHASE B: D = ln(d^2+1), reduce =================
        corr = sm.tile([1, ipc], fp32, tag="corr")
        for img in range(ipc):
            dsq = pk.tile([P, NPACK * CHC], f16, tag="dsq")
            nc.scalar.square(dsq[:], dres[img][:])
            nc.scalar.activation(dsq[:], dsq[:], AF.Ln, bias=1.0)
            nc.vector.tensor_reduce(acc[:, img : img + 1], dsq[:],
                                    axis=mybir.AxisListType.X, op=ALU.add)

            # correction: corr_img = sum_s nd[s] * ln(dv^2+1)
            dv2 = sm.tile([1, S], fp32, tag="dv2")
            nc.scalar.square(dv2[:], dvs[img][:])
            nc.scalar.activation(dv2[:], dv2[:], AF.Ln, bias=1.0)
            nc.vector.tensor_mul(dv2[:], dv2[:], nds[img][:])
            nc.vector.tensor_reduce(corr[:, img : img + 1], dv2[:],
                                    axis=mybir.AxisListType.X, op=ALU.add)

        # ================= finale =================
        accs = sm.tile([P, 1], fp32, tag="accs")
        nc.vector.tensor_add(accs[:], acc[:, 0:1], acc[:, 1:2])
        tot = pst.tile([1, 1], fp32, tag="tot")
        nc.tensor.matmul(tot[:], ones[:], accs[:], start=True, stop=True)
        tots = sm.tile([1, 1], fp32, tag="tots")
        nc.vector.tensor_copy(tots[:], tot[:])
        nc.vector.tensor_sub(tots[:], tots[:], corr[:, 0:1])
        nc.vector.tensor_sub(tots[:], tots[:], corr[:, 1:2])

        # nk = max label of last local image (cross-partition max)
        from concourse import bass_isa
        nkar = sm.tile([P, 1], fp32, tag="nkar")
        nc.gpsimd.partition_all_reduce(nkar[:], nkmax[ipc - 1][:], P,
                                       bass_isa.ReduceOp.max)
        nkf = sm.tile([1, 1], fp32, tag="nkf")
        nc.vector.tensor_copy(nkf[:], nkar[0:1, :])

        outsb = sm.tile([1, 2], fp32, tag="outsb")
        nc.vector.tensor_copy(outsb[:, 0:1], tots[:])
        nc.vector.tensor_copy(outsb[:, 1:2], nkf[:])
        nc.sync.dma_start(out_d.ap(), outsb[:])

    nc.compile()
    return nc


# ================= host-side packing =================

def _prep_image(pred, rm, km, lab, F):
    """pred [C, HW], rm/km [HW] f32, lab [HW] int -> per-image device arrays.

    Groups = 8 contiguous pixel ranges.  Per group: stable-sort by label,
    pack same-label pixels into quads of 4 (dummy slots padded).
    """
    HWi = lab.shape[0]
    gpix = HWi // NG
    FP_ = NG * F // P
    g_of = np.arange(HWi, dtype=np.int64) // gpix
    key = (g_of * 64 + lab).astype(np.int32)
    order = np.argsort(key, kind="stable")
    skey = key[order]
    slab = lab[order].astype(np.int64)
    sg = g_of[order]

    cnt = np.bincount(key, minlength=NG * 64).reshape(NG, 64)
    qcnt = (cnt + 3) // 4
    # quad start (within group) for each (g, s)
    qoff = np.cumsum(qcnt, axis=1) - qcnt
    assert qcnt.sum(axis=1).max() <= F, qcnt.sum(axis=1).max()

    starts = np.cumsum(cnt.reshape(-1)) - cnt.reshape(-1)  # run start per key
    rank = np.arange(HWi, dtype=np.int64) - starts[skey]
    quad = qoff.reshape(-1)[skey] + rank // 4
    slot = rank % 4

    qpix = np.full((NG, F, 4), -1, dtype=np.int64)
    qpix.reshape(-1)[(sg * F + quad) * 4 + slot] = order
    qlab = np.zeros((NG, F), dtype=np.int64)
    qlab.reshape(-1)[sg * F + quad] = slab

    # dummy counts per label (s>=1): quads*4 - real pixels
    pad = qcnt * 4 - cnt
    nd = np.zeros(S, dtype=np.float32)
    nd[1:NLAB] = pad[:, 1:NLAB].sum(axis=0).astype(np.float32)

    mask = qpix >= 0
    qp = np.where(mask, qpix, 0)
    QD = np.zeros((NG, F, 4, 5), dtype=np.float32)
    for c in range(C):
        QD[..., c] = pred[c][qp] * mask
    QD[..., 4] = km[qp] * mask
    QR = rm[qp] * mask  # [NG, F, 4]

    # pdata [128, F]: row 16g+4j+c
    pdata = np.ascontiguousarray(
        QD[..., :4].transpose(0, 2, 3, 1).reshape(P, F)).astype(np.float16)
    # rmQ [128, F]: rm replicated over c
    rmq = np.ascontiguousarray(
        np.broadcast_to(QR[..., None], (NG, F, 4, 4)).transpose(0, 2, 3, 1)
        .reshape(P, F)).astype(np.float16)
    # glab wrapped [128, F//16]
    glab = np.ascontiguousarray(
        qlab.reshape(NG, F // 16, 16).transpose(0, 2, 1).reshape(P, F // 16)
    ).astype(np.uint16)
    # position-major: position P = g*F + i at (p = P%128, q = P//128)
    flat_lab = qlab.reshape(NG * F)
    labpos = np.ascontiguousarray(
        flat_lab.reshape(FP_, P).T).astype(np.float16)
    pdpos = np.ascontiguousarray(
        QD.reshape(NG * F, 20).reshape(FP_, P, 20).transpose(1, 0, 2)
        .reshape(P, 20 * FP_)).astype(np.float16)
    return dict(pdata=pdata, rmq=rmq, glab=glab, labpos=labpos,
                pdpos=pdpos, nd=nd)


def _wall_const():
    w = np.zeros((P, 96), dtype=np.float16)
    p = np.arange(P)
    m = 4 * (p // 16) + (p % 16) // 4
    w[p, m] = 1.0
    w[p, 32 + m] = -2.0
    w[p, 64 + m] = 1.0
    return w


def _wall5_const():
    w = np.zeros((P, 5), dtype=np.float32)
    for j in range(4):
        for c in range(5):
            w[5 * j + c, c] = 1.0
    return w


@functools.lru_cache(maxsize=2)
def _get_full_nc():
    return build_nc(F, CHC, IPC)


def kernel(pred_similarities, regions_mask, kernels_mask, kernel_labels):
    from concourse import bass_utils

    pred = np.asarray(pred_similarities, dtype=np.float32).reshape(B, C, HW)
    rmask = np.asarray(regions_mask, dtype=np.float32).reshape(B, HW)
    km = np.asarray(kernels_mask, dtype=np.float32).reshape(B, HW)
    lab = np.asarray(kernel_labels, dtype=np.int32).reshape(B, HW)

    wall = _wall_const()
    wall5 = _wall5_const()
    in_maps = []
    for i in range(NCORES):
        per_img = [
            _prep_image(pred[i * IPC + j], rmask[i * IPC + j],
                        km[i * IPC + j], lab[i * IPC + j], F)
            for j in range(IPC)
        ]
        in_maps.append({
            "pdpos": np.stack([d["pdpos"].reshape(-1) for d in per_img]),
            "labpos": np.stack([d["labpos"].reshape(-1) for d in per_img]),
            "pdata": np.stack([d["pdata"].reshape(-1) for d in per_img]),
            "rmq": np.stack([d["rmq"].reshape(-1) for d in per_img]),
            "glab": np.stack([d["glab"].reshape(-1) for d in per_img]),
            "nd": np.stack([d["nd"] for d in per_img]),
            "wall": wall.reshape(1, -1),
            "wall5": wall5.reshape(1, -1),
        })

    nc = _get_full_nc()
    res = bass_utils.run_bass_kernel_spmd(nc, in_maps, core_ids=list(range(NCORES)))
    global LAST_RESULT
    LAST_RESULT = res
    outs = [r["out"] for r in res.results]
    total = float(sum(o[0, 0] for o in outs))
    nk = float(outs[NCORES - 1][0, 1])
    return np.array(total / nk, dtype=np.float32)


# ---------------- development helpers ----------------

def _ref_percore(pred, rmask, km, lab):
    """Numpy reference: pred [ipc,C,HW], others [ipc,HW] -> [sum_D, nk]."""
    ipc = lab.shape[0]
    tot = 0.0
    for img in range(ipc):
        x = pred[img].astype(np.float64)
        r = rmask[img].astype(np.float64)
        k_ = km[img].astype(np.float64)
        l_ = lab[img].astype(np.int64)
        sums = np.zeros((64, C))
        card = np.zeros(64)
        np.add.at(card, l_, k_)
        for c in range(C):
            np.add.at(sums[:, c], l_, x[c])
        G = sums / (card[:, None] + 1.0)
        G[0] = 0.0
        g = G[l_]
        fp = x * r[None, :]
        d2 = ((fp.T - g) ** 2).sum(1)
        d = np.maximum(np.sqrt(d2) - SIGMA, 0.0)
        tot += np.log(d * d + 1.0).sum()
    return np.array([tot, lab[ipc - 1].max()], dtype=np.float64)


def _selftest_sim(Ft=256, CHCt=64, hw_t=None):
    from concourse.bass_interp import CoreSim
    rng = np.random.default_rng(0)
    # pick hw so quads fit: worst case quads/group = (hw/8 + 3*37)/4 <= Ft
    if hw_t is None:
        hw_t = (Ft * 4 - 128) * NG  # some slack
    assert hw_t % NG == 0
    ipc = IPC
    pred = rng.standard_normal((ipc, C, hw_t)).astype(np.float32)
    rmask = rng.random((ipc, hw_t)).astype(np.float32)
    km = rng.random((ipc, hw_t)).astype(np.float32)
    lab = rng.integers(0, 37, (ipc, hw_t)).astype(np.int32)

    per_img = [_prep_image(pred[j], rmask[j], km[j], lab[j], Ft)
               for j in range(ipc)]
    nc = build_nc(Ft, CHCt, ipc)
    sim = CoreSim(nc, trace=False)
    sim.tensor("pdpos")[:] = np.stack([d["pdpos"].reshape(-1) for d in per_img])
    sim.tensor("labpos")[:] = np.stack([d["labpos"].reshape(-1) for d in per_img])
    sim.tensor("pdata")[:] = np.stack([d["pdata"].reshape(-1) for d in per_img])
    sim.tensor("rmq")[:] = np.stack([d["rmq"].reshape(-1) for d in per_img])
    sim.tensor("glab")[:] = np.stack([d["glab"].reshape(-1) for d in per_img])
    sim.tensor("nd")[:] = np.stack([d["nd"] for d in per_img])
    sim.tensor("wall")[:] = _wall_const().reshape(1, -1)
    sim.tensor("wall5")[:] = _wall5_const().reshape(1, -1)
    sim.simulate(check_with_hw=False)
    got = np.array(sim.tensor("out")).reshape(2)
    want = _ref_percore(pred, rmask, km, lab)
    print("got ", got)
    print("want", want)
    rel = abs(got[0] - want[0]) / abs(want[0])
    print("rel err:", rel)
    assert got[1] == want[1], (got[1], want[1])
    assert rel < 2e-2, rel
    print("SELFTEST PASS")


def _test_prep():
    """Pure-host check that the packing arrays are consistent."""
    rng = np.random.default_rng(1)
    hw_t = NG * 512
    Ft = 160
    pred = rng.standard_normal((C, hw_t)).astype(np.float32)
    rm = rng.random(hw_t).astype(np.float32)
    km = rng.random(hw_t).astype(np.float32)
    lab = rng.integers(0, 37, hw_t).astype(np.int32)
    d = _prep_image(pred, rm, km, lab, Ft)
    FP_ = NG * Ft // P
    # reconstruct seg sums from pdpos/labpos and compare with direct
    pdpos = d["pdpos"].reshape(P, FP_, 20).astype(np.float64)
    labpos = d["labpos"].astype(np.float64)
    sums = np.zeros((64, 20))
    for s in range(NLAB):
        m = labpos == s
        sums[s] = (pdpos * m[:, :, None]).sum((0, 1))
    ref_sums = np.zeros((64, C))
    ref_card = np.zeros(64)
    for c in range(C):
        np.add.at(ref_sums[:, c], lab, pred[c].astype(np.float64))
    np.add.at(ref_card, lab, km.astype(np.float64))
    got_sums = sums[:, [0, 1, 2, 3]] + sums[:, [5, 6, 7, 8]] \
        + sums[:, [10, 11, 12, 13]] + sums[:, [15, 16, 17, 18]]
    got_card = sums[:, 4] + sums[:, 9] + sums[:, 14] + sums[:, 19]
    print("sums err", np.abs(got_sums - ref_sums).max() / np.abs(ref_sums).max())
    print("card err", np.abs(got_card - ref_card).max() / max(ref_card.max(), 1))
    # check quad-label coherence: pdata rows vs glab
    pdata = d["pdata"].reshape(NG, 4, 4, Ft)
    glab = d["glab"].reshape(NG, 16, Ft // 16)
    qlab = np.zeros((NG, Ft), dtype=np.int64)
    for g in range(NG):
        qlab[g] = glab[g].T.reshape(-1)[
            np.arange(Ft) // 16 * 16 + np.arange(Ft) % 16]  # identity check below
    # glab[g, r, w] = qlab[g, 16w + r] -> invert
    qlab2 = glab.transpose(0, 2, 1).reshape(NG, Ft)
    # labpos consistency: position P = g*Ft+i
    lp2 = d["labpos"].T.reshape(NG * Ft)
    assert (lp2 == qlab2.reshape(-1)).all()
    print("PREP OK")


if __name__ == "__main__":
    _test_prep()
    _selftest_sim()


# revision 15
# speedup vs baseline: 3.9251x; 1.0164x over previous
"""AggregationLoss Trainium2 kernel — quad-stream design.

Host packs each image (per 16-partition group) into a stream of QUADS:
4 same-label pixels per stream position.  Per core: 2 images.

Device per image:
  pass 1 (position-major): one-hot of quad labels [128 pos, 37] ->
     1088 matmuls accumulating [20, 38] = per-(slot,channel) segment sums
     (each MM contracts 128 quads = 512 pixels).
  table: G[c,s] = sums/(card+1), G[:,0]=0 -> gather table tab[128,38]
     (rows 16g+4j+c = G_c), asq[s] = sum_c G_c^2.
  pass 2 (quad-c-row layout): gpsimd indirect_copy gathers tab per quad
     (one index per 4 pixels); n2 = P2 - 2X + A assembled by 3 PSUM-
     accumulated matmuls with constant +-1/-2 weights over c-rows:
       P2 = sum_c Fp^2, X = sum_c Fp*G, A = sum_c G^2  (Fp = pred*rmask)
     then ACT chain: relu-pack -> sqrt -> d = relu(.-sigma), and later
     D = ln(d^2+1) -> reduce.
  dummy slots in partial quads contribute ln(relu(sqrt(asq)-s)^2+1);
  corrected exactly via host-side dummy counts nd[s].
Output per core: [1, 2] = [sum_D, max label of last local image].
"""

import sys
import functools
from contextlib import ExitStack

import numpy as np

for _p in ("/opt/trn_rl_repo",):
    if _p not in sys.path:
        sys.path.insert(0, _p)

# ---- problem constants (hardcoded per contract) ----
B, C, H, W = 16, 4, 736, 736
HW = H * W              # 541696
P = 128
NCORES = 8
IPC = B // NCORES       # images per core = 2
NG = 8                  # partition groups of 16
GPIX = HW // NG         # pixels per group = 67712
NLAB = 37               # labels 0..36
S = 38                  # table width (cols 0..37, col 37 unused)
SIGMA = 0.5

F = 17408               # quad stream length per group (>= 16956 worst case)
CHC = 512               # pass-2 chunk (positions per chunk)
NCH = F // CHC          # 34
FP = (NG * F) // P      # position-major free size = 1088
NPACK = (NCH + 3) // 4  # packed d blocks (9; last half)


def build_nc(F, CHC, ipc):
    import concourse.bass as bass
    import concourse.bacc as bacc
    import concourse.mybir as mybir
    import concourse.tile as tile

    fp32 = mybir.dt.float32
    f16 = mybir.dt.float16
    u16 = mybir.dt.uint16
    AF = mybir.ActivationFunctionType
    ALU = mybir.AluOpType

    NCH = F // CHC
    FP = NG * F // P
    NPACK = (NCH + 3) // 4
    assert F % 64 == 0 and F % CHC == 0 and CHC % 16 == 0
    assert (NG * F // P) % 16 == 0
    assert (NG * F) % P == 0

    nc = bacc.Bacc("TRN2", target_bir_lowering=False, debug=False)

    pdq_d = nc.dram_tensor("pdq", [ipc, P * 32 * FP], f16, kind="ExternalInput")
    labpos_d = nc.dram_tensor("labpos", [ipc, P * FP], f16, kind="ExternalInput")
    pdata_d = nc.dram_tensor("pdata", [ipc, P * F], f16, kind="ExternalInput")
    rmq_d = nc.dram_tensor("rmq", [ipc, P * F], f16, kind="ExternalInput")
    glab_d = nc.dram_tensor("glab", [ipc, P * (F // 16)], u16, kind="ExternalInput")
    nd_d = nc.dram_tensor("nd", [ipc, S], fp32, kind="ExternalInput")
    wall_d = nc.dram_tensor("wall", [1, P * 96], f16, kind="ExternalInput")
    wall5_d = nc.dram_tensor("wall5", [1, P * 5], fp32, kind="ExternalInput")
    out_d = nc.dram_tensor("out", [1, 2], fp32, kind="ExternalOutput")

    pdq_r = pdq_d.ap().rearrange("i (p x) -> i p x", p=P)
    labpos_r = labpos_d.ap().rearrange("i (p x) -> i p x", p=P)
    pdata_r = pdata_d.ap().rearrange("i (p x) -> i p x", p=P)
    rmq_r = rmq_d.ap().rearrange("i (p x) -> i p x", p=P)
    glab_r = glab_d.ap().rearrange("i (p x) -> i p x", p=P)
    wall_r = wall_d.ap().rearrange("i (p x) -> i p x", p=P)

    with tile.TileContext(nc) as tc, ExitStack() as ctx:
        resid = ctx.enter_context(tc.tile_pool(name="resid", bufs=1))
        ohp = ctx.enter_context(tc.tile_pool(name="ohp", bufs=1))
        pdp = ctx.enter_context(tc.tile_pool(name="pdp", bufs=2))
        io = ctx.enter_context(tc.tile_pool(name="io", bufs=3))
        mid = ctx.enter_context(tc.tile_pool(name="mid", bufs=3))
        pk = ctx.enter_context(tc.tile_pool(name="pk", bufs=2))
        ps1 = ctx.enter_context(tc.tile_pool(name="ps1", bufs=2, space="PSUM"))
        ps2 = ctx.enter_context(tc.tile_pool(name="ps2", bufs=2, space="PSUM"))
        pst = ctx.enter_context(tc.tile_pool(name="pst", bufs=1, space="PSUM"))
        sm = ctx.enter_context(tc.tile_pool(name="sm", bufs=4))

        # ---- constants / residents ----
        wall = resid.tile([P, 96], f16, tag="wall")
        nc.sync.dma_start(wall[:], wall_r[0])
        wall5 = resid.tile([P, 5], fp32, tag="wall5")
        nc.sync.dma_start(wall5[:], wall5_d.ap().rearrange("i (p x) -> i p x", p=P)[0])
        ones = resid.tile([P, 1], fp32, tag="ones")
        nc.vector.memset(ones[:], 1.0)
        bneg = resid.tile([P, 1], fp32, tag="bneg")
        nc.vector.memset(bneg[:], -SIGMA)
        acc = resid.tile([P, ipc], fp32, tag="acc")

        tabs, glabs, asqs, dvs, nds, dres, nkmax = [], [], [], [], [], [], []
        for img in range(ipc):
            tabs.append(resid.tile([P, S], f16, name=f"tab{img}", tag=f"tab{img}"))
            glabs.append(resid.tile([P, F // 16], u16, name=f"glab{img}", tag=f"glab{img}"))
            asqs.append(resid.tile([1, S], fp32, name=f"asq{img}", tag=f"asq{img}"))
            dvs.append(resid.tile([1, S], fp32, name=f"dv{img}", tag=f"dv{img}"))
            nds.append(resid.tile([1, S], fp32, name=f"nd{img}", tag=f"nd{img}"))
            dres.append(resid.tile([P, NPACK * CHC], f16, name=f"dres{img}", tag=f"dres{img}"))
            nkmax.append(resid.tile([P, 1], f16, name=f"nk{img}", tag=f"nk{img}"))

        # ================= PASS 1 (both images) =================
        for img in range(ipc):
            lp = ohp.tile([P, FP], f16, tag="labpos")
            nc.sync.dma_start(lp[:], labpos_r[img])
            nc.sync.dma_start(glabs[img][:], glab_r[img])
            nc.sync.dma_start(nds[img][:], nd_d.ap()[img : img + 1, :])

            # one-hot [P, NLAB(+pad to S), FP]
            O = ohp.tile([P, S, FP], f16, tag="onehot")
            for s in range(NLAB):
                nc.vector.tensor_scalar(O[:, s, :], lp[:], float(s), None,
                                        op0=ALU.is_equal)
            nc.vector.memset(O[:, NLAB:S, :], 0.0)

            # max label (for nk) of this image
            nc.vector.tensor_reduce(nkmax[img][:], lp[:],
                                    axis=mybir.AxisListType.X, op=ALU.max)

            psq = ps1.tile([P, 4 * S], fp32, tag="ps1")
            NPD = 4
            NQ = FP // 4
            QDC = NQ // NPD
            for cc in range(NPD):
                pdt = pdp.tile([P, 128 * QDC], f16, tag="pdt")
                nc.sync.dma_start(pdt[:], pdq_r[img, :, 128 * QDC * cc : 128 * QDC * (cc + 1)])
                for ql in range(QDC):
                    Q = cc * QDC + ql
                    rhs = O[:, :, 4 * Q : 4 * Q + 4].rearrange("p s b -> p b s")
                    nc.tensor.matmul(
                        psq[:], pdt[:, 128 * ql : 128 * ql + 128], rhs,
                        start=(Q == 0), stop=(Q == NQ - 1),
                    )

            # ---- table build ----
            # sums5 [5, S] = sum over j of psums[5j+c]: tiny matmul with
            # wall cols 96:101 (W5[5j+c, c] = 1).
            c0 = sm.tile([20, S], fp32, tag="c0")
            nc.vector.tensor_copy(c0[:], psq[0:20, 0:S])
            sa = sm.tile([20, S], fp32, tag="sa")
            nc.vector.tensor_add(sa[:], c0[:], psq[32:52, S : 2 * S])
            c2 = sm.tile([20, S], fp32, tag="c2")
            nc.vector.tensor_copy(c2[:], psq[64:84, 2 * S : 3 * S])
            sb = sm.tile([20, S], fp32, tag="sb")
            nc.vector.tensor_add(sb[:], c2[:], psq[96:116, 3 * S : 4 * S])
            sums20 = sm.tile([20, S], fp32, tag="sums20")
            nc.vector.tensor_add(sums20[:], sa[:], sb[:])
            ps5 = pst.tile([5, S], fp32, tag="ps5")
            nc.tensor.matmul(ps5[:], wall5[0:20, :], sums20[0:20, :],
                             start=True, stop=True)
            sums5 = sm.tile([5, S], fp32, tag="sums5")
            nc.vector.tensor_copy(sums5[:], ps5[:])

            card0 = sm.tile([1, S], fp32, tag="card0")
            nc.scalar.dma_start(card0[:], sums5[4:5, :])
            denom = sm.tile([1, S], fp32, tag="denom")
            nc.vector.tensor_scalar_add(denom[:], card0[:], 1.0)
            recip = sm.tile([1, S], fp32, tag="recip")
            nc.vector.reciprocal(recip[:], denom[:])
            rec4 = sm.tile([4, S], fp32, tag="rec4")
            for c in range(C):
                nc.scalar.dma_start(rec4[c : c + 1, :], recip[:])
            Gf = sm.tile([4, S], fp32, tag="Gf")
            nc.vector.tensor_mul(Gf[:], sums5[0:4, :], rec4[:])
            nc.vector.memset(Gf[:, 0:1], 0.0)
            G16 = sm.tile([4, S], f16, tag="G16")
            nc.vector.tensor_copy(G16[:], Gf[:])

            # asq[s] = sum_c G_c(s)^2 via ones-matmul
            sqG = sm.tile([4, S], fp32, tag="sqG")
            nc.scalar.square(sqG[:], Gf[:])
            psa = pst.tile([1, S], fp32, tag="psa")
            nc.tensor.matmul(psa[:], ones[0:4, :], sqG[:], start=True, stop=True)
            nc.vector.tensor_copy(asqs[img][:], psa[:])

            # tab[16g+4j+c, s] = G_c(s): replicate [4, S] block 32x
            g16 = sm.tile([16, S], f16, tag="g16")
            for r in range(4):
                nc.scalar.dma_start(g16[4 * r : 4 * r + 4, :], G16[:])
            for g in range(NG):
                nc.scalar.dma_start(tabs[img][16 * g : 16 * g + 16, :], g16[:])

        # ================= PASS 2 (both images) =================
        for img in range(ipc):
            packt = None
            for k in range(NCH):
                sl = slice(k * CHC, (k + 1) * CHC)
                gout = io.tile([P, CHC], f16, tag="gout")
                nc.gpsimd.indirect_copy(
                    gout[:], tabs[img][:],
                    glabs[img][:, k * (CHC // 16) : (k + 1) * (CHC // 16)], True)

                pda = io.tile([P, CHC], f16, tag="pda")
                nc.sync.dma_start(pda[:], pdata_r[img, :, sl])
                rmq = io.tile([P, CHC], f16, tag="rmq")
                nc.scalar.dma_start(rmq[:], rmq_r[img, :, sl])

                fpq = mid.tile([P, CHC], f16, tag="fpq")
                nc.vector.tensor_mul(fpq[:], pda[:], rmq[:])
                h = mid.tile([P, CHC], f16, tag="h")
                nc.vector.tensor_sub(h[:], fpq[:], gout[:])
                hsq = mid.tile([P, CHC], f16, tag="hsq")
                nc.scalar.square(hsq[:], h[:])

                n2 = ps2.tile([32, CHC], fp32, tag="n2")
                nc.tensor.matmul(n2[:], wall[:, 0:32], hsq[:], start=True, stop=True)

                # relu-pack into [128, CHC] (4 chunks per pack)
                cc = k % 4
                if cc == 0:
                    packt = pk.tile([P, CHC], f16, tag="packt")
                nc.scalar.activation(packt[32 * cc : 32 * cc + 32, :], n2[:], AF.Relu)
                if cc == 3 or k == NCH - 1:
                    pb = k // 4
                    rows = 32 * (cc + 1)
                    nrm = pk.tile([P, CHC], f16, tag="nrm")
                    nc.scalar.sqrt(nrm[0:rows, :], packt[0:rows, :])
                    nc.scalar.activation(
                        dres[img][0:rows, pb * CHC : (pb + 1) * CHC],
                        nrm[0:rows, :], AF.Relu, bias=bneg[0:rows, :])
                    if rows < P:
                        nc.vector.memset(
                            dres[img][rows:P, pb * CHC : (pb + 1) * CHC], 0.0)

            # dummy-slot correction, sqrt part: dv = relu(sqrt(asq) - sigma)
            sqa = sm.tile([1, S], fp32, tag="sqa")
            nc.scalar.sqrt(sqa[:], asqs[img][:])
            nc.scalar.activation(dvs[img][:], sqa[:], AF.Relu, bias=bneg[0:1, :])

        # ================= PHASE B: D = ln(d^2+1), reduce =================
        corr = sm.tile([1, ipc], fp32, tag="corr")
        for img in range(ipc):
            dsq = pk.tile([P, NPACK * CHC], f16, tag="dsq")
            nc.scalar.square(dsq[:], dres[img][:])
            nc.scalar.activation(dsq[:], dsq[:], AF.Ln, bias=1.0)
            nc.vector.tensor_reduce(acc[:, img : img + 1], dsq[:],
                                    axis=mybir.AxisListType.X, op=ALU.add)

            # correction: corr_img = sum_s nd[s] * ln(dv^2+1)
            dv2 = sm.tile([1, S], fp32, tag="dv2")
            nc.scalar.square(dv2[:], dvs[img][:])
            nc.scalar.activation(dv2[:], dv2[:], AF.Ln, bias=1.0)
            nc.vector.tensor_mul(dv2[:], dv2[:], nds[img][:])
            nc.vector.tensor_reduce(corr[:, img : img + 1], dv2[:],
                                    axis=mybir.AxisListType.X, op=ALU.add)

        # ================= finale =================
        accs = sm.tile([P, 1], fp32, tag="accs")
        nc.vector.tensor_add(accs[:], acc[:, 0:1], acc[:, 1:2])
        tot = pst.tile([1, 1], fp32, tag="tot")
        nc.tensor.matmul(tot[:], ones[:], accs[:], start=True, stop=True)
        tots = sm.tile([1, 1], fp32, tag="tots")
        nc.vector.tensor_copy(tots[:], tot[:])
        nc.vector.tensor_sub(tots[:], tots[:], corr[:, 0:1])
        nc.vector.tensor_sub(tots[:], tots[:], corr[:, 1:2])

        # nk = max label of last local image (cross-partition max)
        from concourse import bass_isa
        nkar = sm.tile([P, 1], fp32, tag="nkar")
        nc.gpsimd.partition_all_reduce(nkar[:], nkmax[ipc - 1][:], P,
                                       bass_isa.ReduceOp.max)
        nkf = sm.tile([1, 1], fp32, tag="nkf")
        nc.vector.tensor_copy(nkf[:], nkar[0:1, :])

        outsb = sm.tile([1, 2], fp32, tag="outsb")
        nc.vector.tensor_copy(outsb[:, 0:1], tots[:])
        nc.vector.tensor_copy(outsb[:, 1:2], nkf[:])
        nc.sync.dma_start(out_d.ap(), outsb[:])

    nc.compile()
    return nc


# ================= host-side packing =================

def _prep_image(pred, rm, km, lab, F):
    """pred [C, HW], rm/km [HW] f32, lab [HW] int -> per-image device arrays.

    Groups = 8 contiguous pixel ranges.  Per group: stable-sort by label,
    pack same-label pixels into quads of 4 (dummy slots padded).
    """
    HWi = lab.shape[0]
    gpix = HWi // NG
    FP_ = NG * F // P
    g_of = np.arange(HWi, dtype=np.int64) // gpix
    key = (g_of * 64 + lab).astype(np.int32)
    order = np.argsort(key, kind="stable")
    skey = key[order]
    slab = lab[order].astype(np.int64)
    sg = g_of[order]

    cnt = np.bincount(key, minlength=NG * 64).reshape(NG, 64)
    qcnt = (cnt + 3) // 4
    # quad start (within group) for each (g, s)
    qoff = np.cumsum(qcnt, axis=1) - qcnt
    assert qcnt.sum(axis=1).max() <= F, qcnt.sum(axis=1).max()

    starts = np.cumsum(cnt.reshape(-1)) - cnt.reshape(-1)  # run start per key
    rank = np.arange(HWi, dtype=np.int64) - starts[skey]
    quad = qoff.reshape(-1)[skey] + rank // 4
    slot = rank % 4

    qpix = np.full((NG, F, 4), -1, dtype=np.int64)
    qpix.reshape(-1)[(sg * F + quad) * 4 + slot] = order
    qlab = np.zeros((NG, F), dtype=np.int64)
    qlab.reshape(-1)[sg * F + quad] = slab

    # dummy counts per label (s>=1): quads*4 - real pixels
    pad = qcnt * 4 - cnt
    nd = np.zeros(S, dtype=np.float32)
    nd[1:NLAB] = pad[:, 1:NLAB].sum(axis=0).astype(np.float32)

    mask = qpix >= 0
    qp = np.where(mask, qpix, 0)
    QD = np.zeros((NG, F, 4, 5), dtype=np.float32)
    for c in range(C):
        QD[..., c] = pred[c][qp] * mask
    QD[..., 4] = km[qp] * mask
    QR = rm[qp] * mask  # [NG, F, 4]

    # pdata [128, F]: row 16g+4j+c
    pdata = np.ascontiguousarray(
        QD[..., :4].transpose(0, 2, 3, 1).reshape(P, F)).astype(np.float16)
    # rmQ [128, F]: rm replicated over c
    rmq = np.ascontiguousarray(
        np.broadcast_to(QR[..., None], (NG, F, 4, 4)).transpose(0, 2, 3, 1)
        .reshape(P, F)).astype(np.float16)
    # glab wrapped [128, F//16]
    glab = np.ascontiguousarray(
        qlab.reshape(NG, F // 16, 16).transpose(0, 2, 1).reshape(P, F // 16)
    ).astype(np.uint16)
    # position-major: position P = g*F + i at (p = P%128, q = P//128)
    flat_lab = qlab.reshape(NG * F)
    labpos = np.ascontiguousarray(
        flat_lab.reshape(FP_, P).T).astype(np.float16)
    arr = QD.reshape(NG * F, 20).reshape(FP_, P, 20)
    tmp = np.zeros((FP_, P, 32), dtype=np.float32)
    tmp[:, :, :20] = arr
    pdq = np.ascontiguousarray(
        tmp.reshape(FP_ // 4, 4, P, 32).transpose(2, 0, 1, 3)
        .reshape(P, 32 * FP_)).astype(np.float16)
    return dict(pdata=pdata, rmq=rmq, glab=glab, labpos=labpos,
                pdq=pdq, nd=nd)


def _wall_const():
    w = np.zeros((P, 96), dtype=np.float16)
    p = np.arange(P)
    m = 4 * (p // 16) + (p % 16) // 4
    w[p, m] = 1.0
    w[p, 32 + m] = -2.0
    w[p, 64 + m] = 1.0
    return w


def _wall5_const():
    w = np.zeros((P, 5), dtype=np.float32)
    for j in range(4):
        for c in range(5):
            w[5 * j + c, c] = 1.0
    return w


@functools.lru_cache(maxsize=2)
def _get_full_nc():
    return build_nc(F, CHC, IPC)


def kernel(pred_similarities, regions_mask, kernels_mask, kernel_labels):
    from concourse import bass_utils

    pred = np.asarray(pred_similarities, dtype=np.float32).reshape(B, C, HW)
    rmask = np.asarray(regions_mask, dtype=np.float32).reshape(B, HW)
    km = np.asarray(kernels_mask, dtype=np.float32).reshape(B, HW)
    lab = np.asarray(kernel_labels, dtype=np.int32).reshape(B, HW)

    wall = _wall_const()
    wall5 = _wall5_const()
    in_maps = []
    for i in range(NCORES):
        per_img = [
            _prep_image(pred[i * IPC + j], rmask[i * IPC + j],
                        km[i * IPC + j], lab[i * IPC + j], F)
            for j in range(IPC)
        ]
        in_maps.append({
            "pdq": np.stack([d["pdq"].reshape(-1) for d in per_img]),
            "labpos": np.stack([d["labpos"].reshape(-1) for d in per_img]),
            "pdata": np.stack([d["pdata"].reshape(-1) for d in per_img]),
            "rmq": np.stack([d["rmq"].reshape(-1) for d in per_img]),
            "glab": np.stack([d["glab"].reshape(-1) for d in per_img]),
            "nd": np.stack([d["nd"] for d in per_img]),
            "wall": wall.reshape(1, -1),
            "wall5": wall5.reshape(1, -1),
        })

    nc = _get_full_nc()
    res = bass_utils.run_bass_kernel_spmd(nc, in_maps, core_ids=list(range(NCORES)))
    global LAST_RESULT
    LAST_RESULT = res
    outs = [r["out"] for r in res.results]
    total = float(sum(o[0, 0] for o in outs))
    nk = float(outs[NCORES - 1][0, 1])
    return np.array(total / nk, dtype=np.float32)


# ---------------- development helpers ----------------

def _ref_percore(pred, rmask, km, lab):
    """Numpy reference: pred [ipc,C,HW], others [ipc,HW] -> [sum_D, nk]."""
    ipc = lab.shape[0]
    tot = 0.0
    for img in range(ipc):
        x = pred[img].astype(np.float64)
        r = rmask[img].astype(np.float64)
        k_ = km[img].astype(np.float64)
        l_ = lab[img].astype(np.int64)
        sums = np.zeros((64, C))
        card = np.zeros(64)
        np.add.at(card, l_, k_)
        for c in range(C):
            np.add.at(sums[:, c], l_, x[c])
        G = sums / (card[:, None] + 1.0)
        G[0] = 0.0
        g = G[l_]
        fp = x * r[None, :]
        d2 = ((fp.T - g) ** 2).sum(1)
        d = np.maximum(np.sqrt(d2) - SIGMA, 0.0)
        tot += np.log(d * d + 1.0).sum()
    return np.array([tot, lab[ipc - 1].max()], dtype=np.float64)


def _selftest_sim(Ft=256, CHCt=64, hw_t=None):
    from concourse.bass_interp import CoreSim
    rng = np.random.default_rng(0)
    # pick hw so quads fit: worst case quads/group = (hw/8 + 3*37)/4 <= Ft
    if hw_t is None:
        hw_t = (Ft * 4 - 128) * NG  # some slack
    assert hw_t % NG == 0
    ipc = IPC
    pred = rng.standard_normal((ipc, C, hw_t)).astype(np.float32)
    rmask = rng.random((ipc, hw_t)).astype(np.float32)
    km = rng.random((ipc, hw_t)).astype(np.float32)
    lab = rng.integers(0, 37, (ipc, hw_t)).astype(np.int32)

    per_img = [_prep_image(pred[j], rmask[j], km[j], lab[j], Ft)
               for j in range(ipc)]
    nc = build_nc(Ft, CHCt, ipc)
    sim = CoreSim(nc, trace=False)
    sim.tensor("pdq")[:] = np.stack([d["pdq"].reshape(-1) for d in per_img])
    sim.tensor("labpos")[:] = np.stack([d["labpos"].reshape(-1) for d in per_img])
    sim.tensor("pdata")[:] = np.stack([d["pdata"].reshape(-1) for d in per_img])
    sim.tensor("rmq")[:] = np.stack([d["rmq"].reshape(-1) for d in per_img])
    sim.tensor("glab")[:] = np.stack([d["glab"].reshape(-1) for d in per_img])
    sim.tensor("nd")[:] = np.stack([d["nd"] for d in per_img])
    sim.tensor("wall")[:] = _wall_const().reshape(1, -1)
    sim.tensor("wall5")[:] = _wall5_const().reshape(1, -1)
    sim.simulate(check_with_hw=False)
    got = np.array(sim.tensor("out")).reshape(2)
    want = _ref_percore(pred, rmask, km, lab)
    print("got ", got)
    print("want", want)
    rel = abs(got[0] - want[0]) / abs(want[0])
    print("rel err:", rel)
    assert got[1] == want[1], (got[1], want[1])
    assert rel < 2e-2, rel
    print("SELFTEST PASS")


def _test_prep():
    """Pure-host check that the packing arrays are consistent."""
    rng = np.random.default_rng(1)
    hw_t = NG * 512
    Ft = 192
    pred = rng.standard_normal((C, hw_t)).astype(np.float32)
    rm = rng.random(hw_t).astype(np.float32)
    km = rng.random(hw_t).astype(np.float32)
    lab = rng.integers(0, 37, hw_t).astype(np.int32)
    d = _prep_image(pred, rm, km, lab, Ft)
    FP_ = NG * Ft // P
    # reconstruct seg sums from pdq/labpos and compare with direct
    pdq = d["pdq"].reshape(P, FP_ // 4, 4, 32).astype(np.float64)
    pdpos = pdq[:, :, :, :20].reshape(P, FP_, 20)
    labpos = d["labpos"].astype(np.float64)
    sums = np.zeros((64, 20))
    for s in range(NLAB):
        m = labpos == s
        sums[s] = (pdpos * m[:, :, None]).sum((0, 1))
    ref_sums = np.zeros((64, C))
    ref_card = np.zeros(64)
    for c in range(C):
        np.add.at(ref_sums[:, c], lab, pred[c].astype(np.float64))
    np.add.at(ref_card, lab, km.astype(np.float64))
    got_sums = sums[:, [0, 1, 2, 3]] + sums[:, [5, 6, 7, 8]] \
        + sums[:, [10, 11, 12, 13]] + sums[:, [15, 16, 17, 18]]
    got_card = sums[:, 4] + sums[:, 9] + sums[:, 14] + sums[:, 19]
    print("sums err", np.abs(got_sums - ref_sums).max() / np.abs(ref_sums).max())
    print("card err", np.abs(got_card - ref_card).max() / max(ref_card.max(), 1))
    # check quad-label coherence: pdata rows vs glab
    pdata = d["pdata"].reshape(NG, 4, 4, Ft)
    glab = d["glab"].reshape(NG, 16, Ft // 16)
    qlab = np.zeros((NG, Ft), dtype=np.int64)
    for g in range(NG):
        qlab[g] = glab[g].T.reshape(-1)[
            np.arange(Ft) // 16 * 16 + np.arange(Ft) % 16]  # identity check below
    # glab[g, r, w] = qlab[g, 16w + r] -> invert
    qlab2 = glab.transpose(0, 2, 1).reshape(NG, Ft)
    # labpos consistency: position P = g*Ft+i
    lp2 = d["labpos"].T.reshape(NG * Ft)
    assert (lp2 == qlab2.reshape(-1)).all()
    print("PREP OK")


if __name__ == "__main__":
    _test_prep()
    _selftest_sim()


# revision 19
# speedup vs baseline: 4.0487x; 1.0315x over previous
"""AggregationLoss Trainium2 kernel — quad-stream design v3.

Host packs each image (per 16-partition group) into a stream of QUADS:
4 same-label pixels per stream position.  Per core: 2 images.

Device per image:
  pass 1 (position-major): one-hot of quad labels [128 pos, 37];
     block-diagonal matmuls: 6 positions per MM (lhsT [128, 6x20+pad],
     rhs = one-hot [128, 38x6] s-outer/b-inner) accumulating [128, 228];
     diagonal blocks recombined with 6 tiny select-matmuls -> [20, 38]
     -> W5 matmul -> per-(channel|km) segment sums [5, 38].
  table: G[c,s] = sums/(card+1), G[:,0]=0 -> gather table tab[128,38]
     (rows 16g+4j+c = G_c), asq[s] = sum_c G_c^2.
  pass 2 (quad-c-row layout): gpsimd indirect_copy gathers tab per quad
     (1024 indices per call, one index per 4 pixels);
     h = pred*rmask - G (f16), n2 = W1 . h^2 (one matmul, sums c-rows);
     ACT: relu-pack [32,512]x4 -> [128,512] -> sqrt -> d = relu(.-sigma);
     later D = ln(d^2+1) -> reduce.  Pass-2 chunks of both images are
     emission-interleaved and packs are delayed to keep all engines busy.
  dummy slots in partial quads contribute ln(relu(sqrt(asq)-sig)^2+1);
  corrected exactly via host-side dummy counts nd[s].
Output per core: [1, 2] = [sum_D, max label of last local image].
"""

import sys
import functools
from contextlib import ExitStack

import numpy as np

for _p in ("/opt/trn_rl_repo",):
    if _p not in sys.path:
        sys.path.insert(0, _p)

# ---- problem constants (hardcoded per contract) ----
B, C, H, W = 16, 4, 736, 736
HW = H * W              # 541696
P = 128
NCORES = 8
IPC = B // NCORES       # images per core = 2
NG = 8                  # partition groups of 16
NLAB = 37               # labels 0..36
S = 38                  # table width (cols 0..37, col 37 unused)
SIGMA = 0.5
BPOS = 6                # positions per pass-1 matmul block

F = 17408               # quad stream length per group (>= 16956 worst case)
CHC = 512               # pass-2 compute chunk (positions)
GCH = 1024              # pass-2 gather/DMA chunk
NCH = F // CHC          # 34
NT = F // GCH           # 17
FP = (NG * F) // P      # raw position columns = 1088
FPQ = ((FP + BPOS - 1) // BPOS) * BPOS   # padded to 1092
NBLK = FPQ // BPOS      # 182
NPACK = (NCH + 3) // 4  # packed d blocks (9; last half)


def _ceil(a, b):
    return (a + b - 1) // b


def build_nc(F, CHC, ipc):
    import concourse.bass as bass
    import concourse.bacc as bacc
    import concourse.mybir as mybir
    import concourse.tile as tile
    from concourse import bass_isa

    fp32 = mybir.dt.float32
    f16 = mybir.dt.float16
    u16 = mybir.dt.uint16
    AF = mybir.ActivationFunctionType
    ALU = mybir.AluOpType

    GCH_ = min(GCH, F)
    NT_ = F // GCH_
    NCH_ = F // CHC
    NPACK_ = (NCH_ + 3) // 4
    FP_ = NG * F // P
    FPQ_ = _ceil(FP_, BPOS) * BPOS
    NBLK_ = FPQ_ // BPOS
    assert F % 16 == 0 and F % CHC == 0 and CHC % 16 == 0 and F % GCH_ == 0

    nc = bacc.Bacc("TRN2", target_bir_lowering=False, debug=False)

    pdq_d = nc.dram_tensor("pdq", [ipc, P * 128 * NBLK_], f16, kind="ExternalInput")
    labpos_d = nc.dram_tensor("labpos", [ipc, P * FPQ_], f16, kind="ExternalInput")
    pdrm_d = nc.dram_tensor("pdrm", [ipc, P * 2 * F], f16, kind="ExternalInput")
    glab_d = nc.dram_tensor("glab", [ipc, P * (F // 16)], u16, kind="ExternalInput")
    nd_d = nc.dram_tensor("nd", [ipc, S], fp32, kind="ExternalInput")
    wall_d = nc.dram_tensor("wall", [1, P * 32], f16, kind="ExternalInput")
    wall5_d = nc.dram_tensor("wall5", [1, P * 5], fp32, kind="ExternalInput")
    wsel_d = nc.dram_tensor("wsel", [1, P * 120], fp32, kind="ExternalInput")
    out_d = nc.dram_tensor("out", [1, 2], fp32, kind="ExternalOutput")

    pdq_r = pdq_d.ap().rearrange("i (p x) -> i p x", p=P)
    labpos_r = labpos_d.ap().rearrange("i (p x) -> i p x", p=P)
    pdrm_r = pdrm_d.ap().rearrange("i (p x) -> i p x", p=P)
    glab_r = glab_d.ap().rearrange("i (p x) -> i p x", p=P)

    with tile.TileContext(nc) as tc, ExitStack() as ctx:
        resid = ctx.enter_context(tc.tile_pool(name="resid", bufs=1))
        ohp = ctx.enter_context(tc.tile_pool(name="ohp", bufs=1))
        pdp = ctx.enter_context(tc.tile_pool(name="pdp", bufs=2))
        io = ctx.enter_context(tc.tile_pool(name="io", bufs=3))
        gop = ctx.enter_context(tc.tile_pool(name="gop", bufs=2))
        mid = ctx.enter_context(tc.tile_pool(name="mid", bufs=4))
        pk = ctx.enter_context(tc.tile_pool(name="pk", bufs=2))
        ps1 = ctx.enter_context(tc.tile_pool(name="ps1", bufs=2, space="PSUM"))
        ps2 = ctx.enter_context(tc.tile_pool(name="ps2", bufs=4, space="PSUM"))
        pst = ctx.enter_context(tc.tile_pool(name="pst", bufs=2, space="PSUM"))
        sm = ctx.enter_context(tc.tile_pool(name="sm", bufs=4))

        # ---- constants / residents ----
        wall = resid.tile([P, 32], f16, tag="wall")
        nc.sync.dma_start(wall[:], wall_d.ap().rearrange("i (p x) -> i p x", p=P)[0])
        wall5 = resid.tile([P, 5], fp32, tag="wall5")
        nc.sync.dma_start(wall5[:], wall5_d.ap().rearrange("i (p x) -> i p x", p=P)[0])
        wsel = resid.tile([P, 120], fp32, tag="wsel")
        nc.sync.dma_start(wsel[:], wsel_d.ap().rearrange("i (p x) -> i p x", p=P)[0])
        ones = resid.tile([P, 1], fp32, tag="ones")
        nc.vector.memset(ones[:], 1.0)
        bneg = resid.tile([P, 1], fp32, tag="bneg")
        nc.vector.memset(bneg[:], -SIGMA)
        acc = resid.tile([P, ipc], fp32, tag="acc")

        tabs, glabs, asqs, dvs, nds, dres, nkmax = [], [], [], [], [], [], []
        for img in range(ipc):
            tabs.append(resid.tile([P, S], f16, name=f"tab{img}", tag=f"tab{img}"))
            glabs.append(resid.tile([P, F // 16], u16, name=f"glab{img}", tag=f"glab{img}"))
            asqs.append(resid.tile([1, S], fp32, name=f"asq{img}", tag=f"asq{img}"))
            dvs.append(resid.tile([1, S], fp32, name=f"dv{img}", tag=f"dv{img}"))
            nds.append(resid.tile([1, S], fp32, name=f"nd{img}", tag=f"nd{img}"))
            dres.append(resid.tile([P, NPACK_ * CHC], f16, name=f"dres{img}", tag=f"dres{img}"))
            nkmax.append(resid.tile([P, 1], f16, name=f"nk{img}", tag=f"nk{img}"))

        st = {}   # per-image pass-1 state

        # ---------- pass-1 pieces ----------
        def emit_A_head(img):
            lp = ohp.tile([P, FPQ_], f16, name="lp", tag="labpos")
            nc.sync.dma_start(lp[:], labpos_r[img])
            nc.sync.dma_start(glabs[img][:], glab_r[img])
            nc.sync.dma_start(nds[img][:], nd_d.ap()[img : img + 1, :])
            O = ohp.tile([P, S, FPQ_], f16, name="O", tag="onehot")
            st[img] = dict(lp=lp, O=O)

        def emit_A_planes(img, s0, s1):
            lp, O = st[img]["lp"], st[img]["O"]
            for s in range(s0, s1):
                nc.vector.tensor_scalar(O[:, s, :], lp[:], float(s), None,
                                        op0=ALU.is_equal)
            if s1 >= NLAB and "nk_done" not in st[img]:
                nc.vector.memset(O[:, NLAB:S, :], 0.0)
                nc.vector.tensor_reduce(nkmax[img][:], lp[:],
                                        axis=mybir.AxisListType.X, op=ALU.max)
                st[img]["nk_done"] = True

        NPD = next(c for c in (7, 4, 2, 1) if NBLK_ % c == 0)
        QDC = NBLK_ // NPD

        def emit_A_mm(img, cc):
            O = st[img]["O"]
            if cc == 0:
                st[img]["psq"] = ps1.tile([P, BPOS * S], fp32, name="psq", tag="psq")
            psq = st[img]["psq"]
            pdt = pdp.tile([P, 128 * QDC], f16, tag="pdt")
            nc.sync.dma_start(pdt[:], pdq_r[img, :, 128 * QDC * cc : 128 * QDC * (cc + 1)])
            for ql in range(QDC):
                Q = cc * QDC + ql
                nc.tensor.matmul(
                    psq[:], pdt[:, 128 * ql : 128 * ql + 128],
                    O[:, :, BPOS * Q : BPOS * (Q + 1)],
                    start=(Q == 0), stop=(Q == NBLK_ - 1),
                )

        def emit_A_table(img):
            psq = st[img]["psq"]
            csq = sm.tile([P, BPOS * S], fp32, tag="csq")
            nc.vector.tensor_copy(csq[:], psq[:])
            csq_r = csq[:].rearrange("p (s b) -> p b s", b=BPOS)
            scr = pst.tile([20, S], fp32, tag="pscratch")
            for u in range(BPOS):
                nc.tensor.matmul(scr[:], wsel[:, 20 * u : 20 * u + 20],
                                 csq_r[:, u : u + 1, :],
                                 start=(u == 0), stop=(u == BPOS - 1))
            sums20 = sm.tile([20, S], fp32, tag="sums20")
            nc.vector.tensor_copy(sums20[:], scr[:])
            nc.tensor.matmul(scr[0:5, :], wall5[0:20, :], sums20[:],
                             start=True, stop=True)
            sums5 = sm.tile([5, S], fp32, tag="sums5")
            nc.vector.tensor_copy(sums5[:], scr[0:5, :])

            card0 = sm.tile([1, S], fp32, tag="card0")
            nc.scalar.dma_start(card0[:], sums5[4:5, :])
            denom = sm.tile([1, S], fp32, tag="denom")
            nc.vector.tensor_scalar_add(denom[:], card0[:], 1.0)
            recip = sm.tile([1, S], fp32, tag="recip")
            nc.vector.reciprocal(recip[:], denom[:])
            rec4 = sm.tile([4, S], fp32, tag="rec4")
            for c in range(C):
                nc.scalar.dma_start(rec4[c : c + 1, :], recip[:])
            Gf = sm.tile([4, S], fp32, tag="Gf")
            nc.vector.tensor_mul(Gf[:], sums5[0:4, :], rec4[:])
            nc.vector.memset(Gf[:, 0:1], 0.0)
            G16 = sm.tile([4, S], f16, tag="G16")
            nc.vector.tensor_copy(G16[:], Gf[:])

            sqG = sm.tile([4, S], fp32, tag="sqG")
            nc.scalar.square(sqG[:], Gf[:])
            nc.tensor.matmul(scr[0:1, :], ones[0:4, :], sqG[:],
                             start=True, stop=True)
            nc.vector.tensor_copy(asqs[img][:], scr[0:1, :])

            g16 = sm.tile([16, S], f16, tag="g16")
            for r in range(4):
                nc.scalar.dma_start(g16[4 * r : 4 * r + 4, :], G16[:])
            for g in range(NG):
                nc.scalar.dma_start(tabs[img][16 * g : 16 * g + 16, :], g16[:])

        # ---------- pass-2 pieces ----------
        packs = {}
        pending = []

        def emit_pack(img, k, n2):
            cc = k % 4
            if cc == 0:
                packs[img] = pk.tile([P, CHC], f16, name=f"packt{img}", tag=f"packt{img}")
            packt = packs[img]
            nc.scalar.activation(packt[32 * cc : 32 * cc + 32, :], n2[:], AF.Relu)
            if cc == 3 or k == NCH_ - 1:
                pb = k // 4
                rows = 32 * (cc + 1)
                nrm = pk.tile([P, CHC], f16, tag="nrm")
                nc.scalar.sqrt(nrm[0:rows, :], packt[0:rows, :])
                nc.scalar.activation(
                    dres[img][0:rows, pb * CHC : (pb + 1) * CHC],
                    nrm[0:rows, :], AF.Relu, bias=bneg[0:rows, :])
                if rows < P:
                    nc.vector.memset(
                        dres[img][rows:P, pb * CHC : (pb + 1) * CHC], 0.0)

        def drain_pending(limit):
            while len(pending) > limit:
                emit_pack(*pending.pop(0))

        def emit_C_slot(img, t):
            pdrm = io.tile([P, 2 * GCH_], f16, tag=f"pdrm{img}")
            nc.sync.dma_start(pdrm[:], pdrm_r[img, :, 2 * GCH_ * t : 2 * GCH_ * (t + 1)])
            gout = gop.tile([P, GCH_], f16, tag=f"gout{img}")
            nc.gpsimd.indirect_copy(
                gout[:], tabs[img][:],
                glabs[img][:, t * (GCH_ // 16) : (t + 1) * (GCH_ // 16)], True)
            for kk in range(GCH_ // CHC):
                k = (GCH_ // CHC) * t + kk
                fpq = mid.tile([P, CHC], f16, tag="fpq")
                nc.vector.tensor_mul(fpq[:], pdrm[:, kk * CHC : (kk + 1) * CHC],
                                     pdrm[:, GCH_ + kk * CHC : GCH_ + (kk + 1) * CHC])
                h = mid.tile([P, CHC], f16, tag="h")
                nc.vector.tensor_sub(h[:], fpq[:], gout[:, kk * CHC : (kk + 1) * CHC])
                hsq = mid.tile([P, CHC], f16, tag="hsq")
                nc.scalar.square(hsq[:], h[:])
                n2 = ps2.tile([32, CHC], fp32, tag="n2")
                nc.tensor.matmul(n2[:], wall[:], hsq[:], start=True, stop=True)
                pending.append((img, k, n2))
            drain_pending(3)

        def emit_C_tail(img):
            sqa = sm.tile([1, S], fp32, tag="sqa")
            nc.scalar.sqrt(sqa[:], asqs[img][:])
            nc.scalar.activation(dvs[img][:], sqa[:], AF.Relu, bias=bneg[0:1, :])

        # ---------- phase B ----------
        corr = sm.tile([1, ipc], fp32, tag="corr")

        def emit_B(img):
            nc.scalar.square(dres[img][:], dres[img][:])
            nc.scalar.activation(dres[img][:], dres[img][:], AF.Ln, bias=1.0)
            nc.vector.tensor_reduce(acc[:, img : img + 1], dres[img][:],
                                    axis=mybir.AxisListType.X, op=ALU.add)
            dv2 = sm.tile([1, S], fp32, tag="dv2")
            nc.scalar.square(dv2[:], dvs[img][:])
            nc.scalar.activation(dv2[:], dv2[:], AF.Ln, bias=1.0)
            nc.vector.tensor_mul(dv2[:], dv2[:], nds[img][:])
            nc.vector.tensor_reduce(corr[:, img : img + 1], dv2[:],
                                    axis=mybir.AxisListType.X, op=ALU.add)

        # ================= emission =================
        emit_A_head(0)
        emit_A_planes(0, 0, NLAB)
        for cc in range(NPD):
            emit_A_mm(0, cc)
        emit_A_table(0)
        emit_A_head(1)

        units = [lambda: emit_A_planes(1, 0, 10),
                 lambda: emit_A_planes(1, 10, 20),
                 lambda: emit_A_planes(1, 20, 30),
                 lambda: emit_A_planes(1, 30, NLAB)]
        units += [(lambda c: (lambda: emit_A_mm(1, c)))(cc) for cc in range(NPD)]
        units.append(lambda: emit_A_table(1))
        NU = len(units)  # 12

        c1_next = 0
        for t in range(NT_):
            emit_C_slot(0, t)
            if t < NU:
                units[t]()
            elif c1_next < NT_:
                emit_C_slot(1, c1_next)
                c1_next += 1
        for u in range(min(NT_, NU), NU):
            units[u]()
        while c1_next < NT_:
            emit_C_slot(1, c1_next)
            c1_next += 1
        drain_pending(0)
        emit_C_tail(0)
        emit_C_tail(1)

        emit_B(0)
        emit_B(1)

        # ================= finale =================
        accs = sm.tile([P, 1], fp32, tag="accs")
        nc.vector.tensor_add(accs[:], acc[:, 0:1], acc[:, 1:2])
        scr2 = pst.tile([20, S], fp32, tag="pscratch")
        nc.tensor.matmul(scr2[0:1, 0:1], ones[:], accs[:], start=True, stop=True)
        tots = sm.tile([1, 1], fp32, tag="tots")
        nc.vector.tensor_copy(tots[:], scr2[0:1, 0:1])
        nc.vector.tensor_sub(tots[:], tots[:], corr[:, 0:1])
        nc.vector.tensor_sub(tots[:], tots[:], corr[:, 1:2])

        from concourse import bass_isa as _bi
        nkar = sm.tile([P, 1], fp32, tag="nkar")
        nc.gpsimd.partition_all_reduce(nkar[:], nkmax[ipc - 1][:], P,
                                       _bi.ReduceOp.max)
        nkf = sm.tile([1, 1], fp32, tag="nkf")
        nc.vector.tensor_copy(nkf[:], nkar[0:1, :])

        outsb = sm.tile([1, 2], fp32, tag="outsb")
        nc.vector.tensor_copy(outsb[:, 0:1], tots[:])
        nc.vector.tensor_copy(outsb[:, 1:2], nkf[:])
        nc.sync.dma_start(out_d.ap(), outsb[:])

    nc.compile()
    return nc


# ================= host-side packing =================

def _prep_image(pred, rm, km, lab, F):
    """pred [C, HW], rm/km [HW] f32, lab [HW] int -> per-image device arrays.

    Groups = 8 contiguous pixel ranges.  Per group: stable-sort by label,
    pack same-label pixels into quads of 4 (dummy slots padded).
    """
    HWi = lab.shape[0]
    gpix = HWi // NG
    FP_ = NG * F // P
    FPQ_ = _ceil(FP_, BPOS) * BPOS
    GCH_ = min(GCH, F)
    g_of = np.arange(HWi, dtype=np.int64) // gpix
    key = (g_of * 64 + lab).astype(np.int32)
    order = np.argsort(key, kind="stable")
    skey = key[order]
    slab = lab[order].astype(np.int64)
    sg = g_of[order]

    cnt = np.bincount(key, minlength=NG * 64).reshape(NG, 64)
    qcnt = (cnt + 3) // 4
    qoff = np.cumsum(qcnt, axis=1) - qcnt
    assert qcnt.sum(axis=1).max() <= F, qcnt.sum(axis=1).max()

    starts = np.cumsum(cnt.reshape(-1)) - cnt.reshape(-1)
    rank = np.arange(HWi, dtype=np.int64) - starts[skey]
    quad = qoff.reshape(-1)[skey] + rank // 4
    slot = rank % 4

    qpix = np.full((NG, F, 4), -1, dtype=np.int64)
    qpix.reshape(-1)[(sg * F + quad) * 4 + slot] = order
    qlab = np.zeros((NG, F), dtype=np.int64)
    qlab.reshape(-1)[sg * F + quad] = slab

    pad = qcnt * 4 - cnt
    nd = np.zeros(S, dtype=np.float32)
    nd[1:NLAB] = pad[:, 1:NLAB].sum(axis=0).astype(np.float32)

    mask = qpix >= 0
    qp = np.where(mask, qpix, 0)
    QD = np.zeros((NG, F, 4, 5), dtype=np.float32)
    for c in range(C):
        QD[..., c] = pred[c][qp] * mask
    QD[..., 4] = km[qp] * mask
    QR = rm[qp] * mask  # [NG, F, 4]

    # pdata [128, F]: row 16g+4j+c ; rmQ replicated over c
    pdata = np.ascontiguousarray(
        QD[..., :4].transpose(0, 2, 3, 1).reshape(P, F)).astype(np.float16)
    rmq = np.ascontiguousarray(
        np.broadcast_to(QR[..., None], (NG, F, 4, 4)).transpose(0, 2, 3, 1)
        .reshape(P, F)).astype(np.float16)
    # merged [pdata | rmq] per GCH block
    pdrm = np.empty((P, 2 * F), dtype=np.float16)
    v = pdrm.reshape(P, F // GCH_, 2, GCH_)
    v[:, :, 0, :] = pdata.reshape(P, F // GCH_, GCH_)
    v[:, :, 1, :] = rmq.reshape(P, F // GCH_, GCH_)

    glab = np.ascontiguousarray(
        qlab.reshape(NG, F // 16, 16).transpose(0, 2, 1).reshape(P, F // 16)
    ).astype(np.uint16)

    # position-major (padded to FPQ_ cols): position P = g*F + i at
    # (p = P%128, q = P//128)
    npos = FPQ_ * P
    flat_lab = np.zeros(npos, dtype=np.int64)
    flat_lab[: NG * F] = qlab.reshape(NG * F)
    labpos = np.ascontiguousarray(
        flat_lab.reshape(FPQ_, P).T).astype(np.float16)

    flat_qd = np.zeros((npos, 20), dtype=np.float32)
    flat_qd[: NG * F] = QD.reshape(NG * F, 20)
    arr = flat_qd.reshape(FPQ_, P, 20)
    blk = np.zeros((FPQ_ // BPOS, BPOS, P, 32), dtype=np.float32)
    blk[..., :20] = arr.reshape(FPQ_ // BPOS, BPOS, P, 20)
    # pdq[p, 128*Q + 20*u + r] = arr[BPOS*Q + u, p, r]
    pdq = np.zeros((FPQ_ // BPOS, P, 128), dtype=np.float32)
    for u in range(BPOS):
        pdq[:, :, 20 * u : 20 * u + 20] = blk[:, u, :, :20]
    pdq = np.ascontiguousarray(
        pdq.transpose(1, 0, 2).reshape(P, 128 * (FPQ_ // BPOS))).astype(np.float16)
    return dict(pdrm=pdrm, glab=glab, labpos=labpos, pdq=pdq, nd=nd)


def _wall_const():
    w = np.zeros((P, 32), dtype=np.float16)
    p = np.arange(P)
    m = 4 * (p // 16) + (p % 16) // 4
    w[p, m] = 1.0
    return w


def _wall5_const():
    w = np.zeros((P, 5), dtype=np.float32)
    for j in range(4):
        for c in range(5):
            w[5 * j + c, c] = 1.0
    return w


def _wsel_const():
    w = np.zeros((P, 120), dtype=np.float32)
    for u in range(BPOS):
        for r in range(20):
            w[20 * u + r, 20 * u + r] = 1.0
    return w


@functools.lru_cache(maxsize=2)
def _get_full_nc():
    return build_nc(F, CHC, IPC)


def kernel(pred_similarities, regions_mask, kernels_mask, kernel_labels):
    from concourse import bass_utils

    pred = np.asarray(pred_similarities, dtype=np.float32).reshape(B, C, HW)
    rmask = np.asarray(regions_mask, dtype=np.float32).reshape(B, HW)
    km = np.asarray(kernels_mask, dtype=np.float32).reshape(B, HW)
    lab = np.asarray(kernel_labels, dtype=np.int32).reshape(B, HW)

    wall = _wall_const()
    wall5 = _wall5_const()
    wsel = _wsel_const()
    in_maps = []
    for i in range(NCORES):
        per_img = [
            _prep_image(pred[i * IPC + j], rmask[i * IPC + j],
                        km[i * IPC + j], lab[i * IPC + j], F)
            for j in range(IPC)
        ]
        in_maps.append({
            "pdq": np.stack([d["pdq"].reshape(-1) for d in per_img]),
            "labpos": np.stack([d["labpos"].reshape(-1) for d in per_img]),
            "pdrm": np.stack([d["pdrm"].reshape(-1) for d in per_img]),
            "glab": np.stack([d["glab"].reshape(-1) for d in per_img]),
            "nd": np.stack([d["nd"] for d in per_img]),
            "wall": wall.reshape(1, -1),
            "wall5": wall5.reshape(1, -1),
            "wsel": wsel.reshape(1, -1),
        })

    nc = _get_full_nc()
    res = bass_utils.run_bass_kernel_spmd(nc, in_maps, core_ids=list(range(NCORES)))
    global LAST_RESULT
    LAST_RESULT = res
    outs = [r["out"] for r in res.results]
    total = float(sum(o[0, 0] for o in outs))
    nk = float(outs[NCORES - 1][0, 1])
    return np.array(total / nk, dtype=np.float32)


# ---------------- development helpers ----------------

def _ref_percore(pred, rmask, km, lab):
    ipc = lab.shape[0]
    tot = 0.0
    for img in range(ipc):
        x = pred[img].astype(np.float64)
        r = rmask[img].astype(np.float64)
        k_ = km[img].astype(np.float64)
        l_ = lab[img].astype(np.int64)
        sums = np.zeros((64, C))
        card = np.zeros(64)
        np.add.at(card, l_, k_)
        for c in range(C):
            np.add.at(sums[:, c], l_, x[c])
        G = sums / (card[:, None] + 1.0)
        G[0] = 0.0
        g = G[l_]
        fp = x * r[None, :]
        d2 = ((fp.T - g) ** 2).sum(1)
        d = np.maximum(np.sqrt(d2) - SIGMA, 0.0)
        tot += np.log(d * d + 1.0).sum()
    return np.array([tot, lab[ipc - 1].max()], dtype=np.float64)


def _selftest_sim(Ft=1024, CHCt=256, hw_t=None):
    from concourse.bass_interp import CoreSim
    rng = np.random.default_rng(0)
    if hw_t is None:
        hw_t = (Ft * 4 - 128) * NG
    assert hw_t % NG == 0
    ipc = IPC
    pred = rng.standard_normal((ipc, C, hw_t)).astype(np.float32)
    rmask = rng.random((ipc, hw_t)).astype(np.float32)
    km = rng.random((ipc, hw_t)).astype(np.float32)
    lab = rng.integers(0, 37, (ipc, hw_t)).astype(np.int32)

    per_img = [_prep_image(pred[j], rmask[j], km[j], lab[j], Ft)
               for j in range(ipc)]
    nc = build_nc(Ft, CHCt, ipc)
    sim = CoreSim(nc, trace=False)
    sim.tensor("pdq")[:] = np.stack([d["pdq"].reshape(-1) for d in per_img])
    sim.tensor("labpos")[:] = np.stack([d["labpos"].reshape(-1) for d in per_img])
    sim.tensor("pdrm")[:] = np.stack([d["pdrm"].reshape(-1) for d in per_img])
    sim.tensor("glab")[:] = np.stack([d["glab"].reshape(-1) for d in per_img])
    sim.tensor("nd")[:] = np.stack([d["nd"] for d in per_img])
    sim.tensor("wall")[:] = _wall_const().reshape(1, -1)
    sim.tensor("wall5")[:] = _wall5_const().reshape(1, -1)
    sim.tensor("wsel")[:] = _wsel_const().reshape(1, -1)
    sim.simulate(check_with_hw=False)
    got = np.array(sim.tensor("out")).reshape(2)
    want = _ref_percore(pred, rmask, km, lab)
    print("got ", got)
    print("want", want)
    rel = abs(got[0] - want[0]) / abs(want[0])
    print("rel err:", rel)
    assert got[1] == want[1], (got[1], want[1])
    assert rel < 2e-2, rel
    print("SELFTEST PASS")


if __name__ == "__main__":
    _selftest_sim()
